# revision 17
# baseline (speedup 1.0000x reference)
"""Multi-head scaled-cosine attention (B=2, L=2048, E=2048, H=16, D=128) on 8 trn2 cores.

Sharding: core c = (b, g) with b = batch (2), g = head-group of 4 heads (4 groups).
Each core computes its 4 heads' attention for its batch plus the partial output
projection; the host sums the 4 per-group partials per batch.

Precision: matmuls run in bf16 (PSUM accumulation is fp32); the softmax
denominator is accumulated in float32r and reduced across partitions with a
ones-matmul. The Q/K RMS-norm cancels exactly under the subsequent L2
normalization; the L2 reciprocal (and logit scale) are folded into a diagonal
matrix applied by the PE transpose that produces Q^T/K^T. exp(bias - rowmax) is
precomputed on the host and folded in multiplicatively. Scores are built
directly in [k, q] orientation so softmax and attn@V need no on-chip transpose
of the probability matrix. Q/K head dims are host-permuted (evens|odds) so RoPE
uses contiguous vector ops; the permutation cancels inside q.k.
"""
import sys
sys.path.insert(0, '/opt/trn_rl_repo')
import math
import numpy as np
import ml_dtypes

import concourse.bacc as bacc
import concourse.mybir as mybir
import concourse.tile as tile
from concourse.bass_utils import run_bass_kernel_spmd

F32 = mybir.dt.float32
F32R = mybir.dt.float32r
BF16 = mybir.dt.bfloat16
NP_BF16 = ml_dtypes.bfloat16
ALU = mybir.AluOpType
AF = mybir.ActivationFunctionType

B, L, E, H, D = 2, 2048, 2048, 16, 128
G = 4                 # head groups
HPG = H // G          # heads per group = 4
GD = HPG * D          # 512, per-group projection width
P = 128               # partitions
NLT = L // P          # 16 l-tiles
NET = E // P          # 16 e-tiles (contraction)
NQC = L // 512        # 4 q-chunks
NKT = L // P          # 16 k-tiles
HD2 = GD // 2         # 256
LOGIT_SCALE_MAX = math.log(1.0 / 0.01)


def _build(apply_qs: bool, apply_ks: bool):
    nc = bacc.Bacc(None, target_bir_lowering=False)
    d = {}
    d['xqT'] = nc.dram_tensor("xqT", [E, L], BF16, kind="ExternalInput")
    d['xkvT'] = nc.dram_tensor("xkvT", [E, L], BF16, kind="ExternalInput")
    d['expBT'] = nc.dram_tensor("expBT", [L, L], BF16, kind="ExternalInput")
    d['wqT'] = nc.dram_tensor("wqT", [E, GD], BF16, kind="ExternalInput")
    d['wkT'] = nc.dram_tensor("wkT", [E, GD], BF16, kind="ExternalInput")
    d['wvT'] = nc.dram_tensor("wvT", [E, GD], BF16, kind="ExternalInput")
    d['woS'] = nc.dram_tensor("woS", [GD, E], BF16, kind="ExternalInput")
    d['c4q'] = nc.dram_tensor("c4q", [L, HD2], F32, kind="ExternalInput")
    d['s4q'] = nc.dram_tensor("s4q", [L, HD2], F32, kind="ExternalInput")
    d['c4k'] = nc.dram_tensor("c4k", [L, HD2], F32, kind="ExternalInput")
    d['s4k'] = nc.dram_tensor("s4k", [L, HD2], F32, kind="ExternalInput")
    d['ls'] = nc.dram_tensor("ls", [P, HPG], F32, kind="ExternalInput")
    if apply_qs:
        d['qscale'] = nc.dram_tensor("qscale", [P, GD], F32, kind="ExternalInput")
    if apply_ks:
        d['kscale'] = nc.dram_tensor("kscale", [P, GD], F32, kind="ExternalInput")
    out = nc.dram_tensor("out", [L, E], F32, kind="ExternalOutput")

    with tile.TileContext(nc) as tc:
        with tc.tile_pool(name="persist", bufs=1) as persist:
            qT = [persist.tile([P, L], BF16, tag=f"qT{h}", name=f"qT{h}") for h in range(HPG)]
            kT = [persist.tile([P, L], BF16, tag=f"kT{h}", name=f"kT{h}") for h in range(HPG)]
            v_sb = persist.tile([P, NLT, GD], BF16, tag="v_sb")
            identb = persist.tile([P, P], BF16, tag="identb")
            identf = persist.tile([P, P], F32, tag="identf")
            nc.vector.memset(identf[:], 0.0)
            nc.gpsimd.affine_select(out=identf[:], in_=identf[:],
                                    compare_op=ALU.not_equal, fill=1.0, base=0,
                                    pattern=[[-1, P]], channel_multiplier=1)
            nc.vector.tensor_copy(identb[:], identf[:])
            ones_f = persist.tile([P, P], F32, tag="ones_f")
            nc.vector.memset(ones_f[:], 1.0)
            ones_r = persist.tile([P, P], F32R, tag="ones_r")
            nc.scalar.copy(ones_r[:], ones_f[:])
            ones_b = persist.tile([P, P], BF16, tag="ones_b")
            nc.vector.tensor_copy(ones_b[:], ones_f[:])
            ls_t = persist.tile([P, HPG], F32, tag="ls_t")
            nc.sync.dma_start(ls_t[:], d['ls'][:])
            w_all = {}
            for wname in ('wvT', 'wkT', 'wqT'):
                w_all[wname] = persist.tile([P, NET, GD], BF16, tag=wname, name=f"w_{wname}")
                nc.sync.dma_start(
                    w_all[wname][:], d[wname][:].rearrange("(e p) n -> p e n", p=P))
            wo_sb = persist.tile([P, HPG, E], BF16, tag="wo_sb")
            nc.sync.dma_start(
                wo_sb[:], d['woS'][:].rearrange("(h p) e -> p h e", p=P))

            qs_t = ks_t = None
            if apply_qs:
                qs_t = persist.tile([P, GD], F32, tag="qs_t")
                nc.sync.dma_start(qs_t[:], d['qscale'][:])
            if apply_ks:
                ks_t = persist.tile([P, GD], F32, tag="ks_t")
                nc.sync.dma_start(ks_t[:], d['kscale'][:])

            from contextlib import ExitStack
            proj_ctx = ExitStack()
            sbp = proj_ctx.enter_context(tc.tile_pool(name="proj_sb", bufs=3))
            nrm = proj_ctx.enter_context(tc.tile_pool(name="proj_nrm", bufs=3))
            psp = proj_ctx.enter_context(tc.tile_pool(name="proj_ps", bufs=2, space="PSUM"))
            pst = proj_ctx.enter_context(tc.tile_pool(name="proj_pst", bufs=2, space="PSUM"))

            def proj_tile(lt, x_dram, w_sb):
                blk = sbp.tile([P, NET, P], BF16, tag="xblk", name=f"xblk_{lt}")
                nc.sync.dma_start(
                    blk[:],
                    x_dram[:, lt * P:(lt + 1) * P].rearrange("(g p) l -> p g l", p=P))
                psum = psp.tile([P, GD], F32, tag="psum", name=f"psum_{lt}")
                for e in range(NET):
                    nc.tensor.matmul(psum[:], blk[:, e, :], w_sb[:, e, :],
                                     start=(e == 0), stop=(e == NET - 1))
                return psum

            def qk_norm(lt, psum, c_dram, s_dram, scale_tile, use_ls, dstT):
                q1 = nrm.tile([P, GD], BF16, tag="q1")
                nc.scalar.copy(q1[:], psum[:])
                if scale_tile is not None:
                    nc.vector.tensor_mul(q1[:], q1[:], scale_tile[:])
                ct = nrm.tile([P, HD2], F32, tag="ct")
                st = nrm.tile([P, HD2], F32, tag="st")
                nc.sync.dma_start(ct[:], c_dram[lt * P:(lt + 1) * P, :])
                nc.sync.dma_start(st[:], s_dram[lt * P:(lt + 1) * P, :])
                # per-head layout [evens(64) | odds(64)] (host-permuted weights)
                q1v = q1[:].rearrange("p (hh par dd) -> p hh par dd", hh=HPG, par=2)
                qe, qo = q1v[:, :, 0, :], q1v[:, :, 1, :]
                q2 = nrm.tile([P, GD], BF16, tag="q2")
                q2v = q2[:].rearrange("p (hh par dd) -> p hh par dd", hh=HPG, par=2)
                re, ro = q2v[:, :, 0, :], q2v[:, :, 1, :]
                ctv = ct[:].rearrange("p (hh dd) -> p hh dd", hh=HPG)
                stv = st[:].rearrange("p (hh dd) -> p hh dd", hh=HPG)
                tmp = nrm.tile([P, HD2], BF16, tag="tmp")
                tv = tmp[:].rearrange("p (hh dd) -> p hh dd", hh=HPG)
                # evens: qe*c - qo*s ; odds: qo*c + qe*s
                nc.vector.tensor_tensor(tv, qo, stv, ALU.mult)
                nc.vector.tensor_tensor(re, qe, ctv, ALU.mult)
                nc.vector.tensor_sub(re, re, tv)
                nc.vector.tensor_tensor(tv, qe, stv, ALU.mult)
                nc.vector.tensor_tensor(ro, qo, ctv, ALU.mult)
                nc.vector.tensor_add(ro, ro, tv)
                # L2 norm over each head's (now contiguous) D slice
                sqs = nrm.tile([P, GD], BF16, tag="sqs")
                acc = nrm.tile([P, HPG], F32, tag="acc")
                for h in range(HPG):
                    nc.scalar.activation(sqs[:, h * D:(h + 1) * D], q2[:, h * D:(h + 1) * D],
                                         AF.Square, accum_out=acc[:, h:h + 1])
                nrm_t = nrm.tile([P, HPG], F32, tag="nrm_t")
                nc.scalar.activation(nrm_t[:], acc[:], AF.Sqrt)
                nc.vector.tensor_scalar_max(nrm_t[:], nrm_t[:], 1e-12)
                rcp = nrm.tile([P, HPG], F32, tag="rcp")
                nc.vector.reciprocal(rcp[:], nrm_t[:])
                if use_ls:
                    nc.vector.tensor_mul(rcp[:], rcp[:], ls_t[:])
                q3 = nrm.tile([P, GD], BF16, tag="q3")
                for h in range(HPG):
                    nc.vector.tensor_scalar_mul(q3[:, h * D:(h + 1) * D],
                                                q2[:, h * D:(h + 1) * D], rcp[:, h:h + 1])
                for h in range(HPG):
                    pt = pst.tile([P, P], BF16, tag="pt", name=f"pt_{lt}_{h}")
                    nc.tensor.matmul(pt[:], q3[:, h * D:(h + 1) * D], identb[:],
                                     is_transpose=True)
                    nc.any.tensor_copy(dstT[h][:, lt * P:(lt + 1) * P], pt[:])

            # V and K phases over all l-tiles
            for lt in range(NLT):
                psum = proj_tile(lt, d['xkvT'], w_all['wvT'])
                nc.scalar.copy(v_sb[:, lt, :], psum[:])
            for lt in range(NLT):
                psum = proj_tile(lt, d['xkvT'], w_all['wkT'])
                qk_norm(lt, psum, d['c4k'], d['s4k'], ks_t, False, kT)

            for lt in range(NLT):
                psum = proj_tile(lt, d['xqT'], w_all['wqT'])
                qk_norm(lt, psum, d['c4q'], d['s4q'], qs_t, True, qT)
            proj_ctx.close()

            # attention per q-chunk
            att_ctx = ExitStack()
            asb = att_ctx.enter_context(tc.tile_pool(name="att_sb", bufs=3))
            atp = att_ctx.enter_context(tc.tile_pool(name="att_at", bufs=1))
            aop = att_ctx.enter_context(tc.tile_pool(name="att_o", bufs=3))
            ps_pv = att_ctx.enter_context(tc.tile_pool(name="ps_pv", bufs=1, space="PSUM"))
            ps_s = att_ctx.enter_context(tc.tile_pool(name="ps_s", bufs=3, space="PSUM"))
            ps_d = att_ctx.enter_context(tc.tile_pool(name="ps_d", bufs=1, space="PSUM"))
            for qc in range(NQC):
                qsl = slice(qc * 512, (qc + 1) * 512)
                pv = [ps_pv.tile([P, 512], F32, tag=f"pv{h}", name=f"pv{qc}_{h}")
                      for h in range(HPG)]
                den = ps_d.tile([P, 512], F32, tag="den", name=f"den{qc}")
                for kt in range(NKT):
                    eb = asb.tile([P, 512], BF16, tag="eb", name=f"eb{qc}_{kt}")
                    nc.sync.dma_start(eb[:], d['expBT'][kt * P:(kt + 1) * P, qsl])
                    p_t = asb.tile([P, 2048], BF16, tag="p_t", name=f"pt{qc}_{kt}", bufs=3)
                    ebb = eb[:].rearrange("p (o q) -> p o q", o=1).broadcast_to([P, 2, 512])
                    for hp in range(2):
                        for i in range(2):
                            h = 2 * hp + i
                            s_ps = ps_s.tile([P, 512], F32, tag="s_ps", name=f"sp{qc}_{kt}_{h}")
                            nc.tensor.matmul(s_ps[:], kT[h][:, kt * P:(kt + 1) * P],
                                             qT[h][:, qsl], start=True, stop=True)
                            nc.scalar.activation(p_t[:, h * 512:(h + 1) * 512], s_ps[:], AF.Exp)
                        half = p_t[:, hp * 1024:(hp + 1) * 1024].rearrange("p (i q) -> p i q", i=2)
                        nc.vector.tensor_tensor(half, half, ebb, ALU.mult)
                        for i in range(2):
                            h = 2 * hp + i
                            nc.tensor.matmul(pv[h][:], v_sb[:, kt, h * D:(h + 1) * D],
                                             p_t[:, h * 512:(h + 1) * 512],
                                             start=(kt == 0), stop=(kt == NKT - 1))
                        for i in range(2):
                            h = 2 * hp + i
                            nc.tensor.matmul(den[32 * h:32 * h + 32, :],
                                             ones_b[:, 0:32],
                                             p_t[:, h * 512:(h + 1) * 512],
                                             start=(kt == 0), stop=(kt == NKT - 1),
                                             tile_position=(0, 32 * h))
                attn = [atp.tile([P, 512], BF16, tag=f"at{h}", name=f"at{qc}_{h}")
                        for h in range(HPG)]
                pvc = []
                for h in range(HPG):
                    c = asb.tile([P, 512], F32, tag=f"pvc{h}", name=f"pvc{qc}_{h}", bufs=2)
                    nc.vector.tensor_copy(c[:], pv[h][:])
                    pvc.append(c)
                rcp32s = []
                for i in range(2):
                    lnd = asb.tile([64, 512], F32, tag=f"lnd{i}", name=f"lnd{qc}_{i}")
                    nc.scalar.activation(lnd[:], den[64 * i:64 * i + 64, :], AF.Ln)
                    rcp32 = asb.tile([64, 512], F32R, tag=f"rcp32{i}", name=f"rcp32{qc}_{i}")
                    nc.scalar.activation(rcp32[:], lnd[:], AF.Exp, scale=-1.0)
                    rcp32s.append(rcp32)
                for h in range(HPG):
                    b_ps = ps_s.tile([P, 512], F32, tag="s_ps", name=f"b{qc}_{h}")
                    r32 = rcp32s[h // 2]
                    off = 32 * (h % 2)
                    nc.tensor.matmul(b_ps[:], ones_r[off:off + 1, :],
                                     r32[off:off + 1, :], start=True, stop=True)
                    rcpb = asb.tile([P, 512], BF16, tag="rcpb", name=f"rb{qc}_{h}")
                    nc.vector.tensor_copy(rcpb[:], b_ps[:])
                    nc.vector.tensor_mul(attn[h][:], pvc[h][:], rcpb[:])
                for lsub in range(4):
                    for ec in range(4):
                        o_ps = ps_s.tile([P, 512], F32, tag="s_ps", name=f"o{qc}_{lsub}_{ec}")
                        for h in range(HPG):
                            nc.tensor.matmul(o_ps[:], attn[h][:, lsub * P:(lsub + 1) * P],
                                             wo_sb[:, h, ec * 512:(ec + 1) * 512],
                                             start=(h == 0), stop=(h == HPG - 1))
                        o_sb = aop.tile([P, 512], F32, tag="o_sb", name=f"ob{qc}_{lsub}_{ec}")
                        nc.vector.tensor_copy(o_sb[:], o_ps[:])
                        nc.sync.dma_start(
                            out[qc * 512 + lsub * P: qc * 512 + (lsub + 1) * P,
                                ec * 512:(ec + 1) * 512], o_sb[:])
            att_ctx.close()
    nc.compile()
    return nc


# head-dim permutation: within each head, evens first then odds
_PERM = np.empty(GD, np.int64)
for _i in range(GD):
    _h, _j = divmod(_i, D)
    _par, _dd = divmod(_j, D // 2)
    _PERM[_i] = _h * D + 2 * _dd + _par


def _prepare(inputs):
    f32 = np.float32
    inputs_q = np.asarray(inputs["inputs_q"], f32)
    inputs_kv = np.asarray(inputs["inputs_kv"], f32)
    bias = np.asarray(inputs["bias"], f32).reshape(L, L)
    q_sin = np.asarray(inputs["q_sinusoids"], f32)
    k_sin = np.asarray(inputs["k_sinusoids"], f32)
    Wq = np.asarray(inputs["Wq"], f32)
    Wk = np.asarray(inputs["Wk"], f32)
    Wv = np.asarray(inputs["Wv"], f32)
    Wo = np.asarray(inputs["Wo"], f32)
    qns = np.asarray(inputs["q_norm_scale"], f32)
    kns = np.asarray(inputs["k_norm_scale"], f32)
    ls = np.asarray(inputs["logit_scale"], f32)

    apply_qs = not np.all(qns == 1.0)
    apply_ks = not np.all(kns == 1.0)

    bm = bias.max(axis=1, keepdims=True)
    expBT = np.ascontiguousarray(np.exp((bias - bm).T).astype(NP_BF16))
    ls_e = np.exp(np.minimum(ls, LOGIT_SCALE_MAX)).astype(f32)

    per_b = []
    for b in range(B):
        per_b.append(dict(
            xqT=np.ascontiguousarray(inputs_q[b].T.astype(NP_BF16)),
            xkvT=np.ascontiguousarray(inputs_kv[b].T.astype(NP_BF16)),
            c4q=np.ascontiguousarray(np.tile(q_sin[b][:, 0::2], (1, HPG))),
            s4q=np.ascontiguousarray(np.tile(q_sin[b][:, 1::2], (1, HPG))),
            c4k=np.ascontiguousarray(np.tile(k_sin[b][:, 0::2], (1, HPG))),
            s4k=np.ascontiguousarray(np.tile(k_sin[b][:, 1::2], (1, HPG))),
        ))
    per_g = []
    for g in range(G):
        rows = slice(g * GD, (g + 1) * GD)
        per_g.append(dict(
            wqT=np.ascontiguousarray(Wq[rows, :][_PERM, :].T.astype(NP_BF16)),
            wkT=np.ascontiguousarray(Wk[rows, :][_PERM, :].T.astype(NP_BF16)),
            wvT=np.ascontiguousarray(Wv[rows, :].T.astype(NP_BF16)),
            woS=np.ascontiguousarray(Wo[:, rows].T.astype(NP_BF16)),
            ls=np.broadcast_to(ls_e[g * HPG:(g + 1) * HPG][None, :], (P, HPG)).copy(),
        ))

    qs_bc = (np.broadcast_to(np.tile(qns, HPG)[_PERM][None, :], (P, GD)).copy()
             if apply_qs else None)
    ks_bc = (np.broadcast_to(np.tile(kns, HPG)[_PERM][None, :], (P, GD)).copy()
             if apply_ks else None)

    in_maps = []
    for c in range(8):
        b, g = divmod(c, G)
        m = dict(expBT=expBT)
        m.update(per_b[b])
        m.update(per_g[g])
        if apply_qs:
            m['qscale'] = qs_bc
        if apply_ks:
            m['kscale'] = ks_bc
        in_maps.append(m)
    return in_maps, apply_qs, apply_ks


_CACHE = {}


def _get_nc(apply_qs, apply_ks):
    key = (apply_qs, apply_ks)
    if key not in _CACHE:
        _CACHE[key] = _build(apply_qs, apply_ks)
    return _CACHE[key]


def kernel(**inputs) -> np.ndarray:
    in_maps, apply_qs, apply_ks = _prepare(inputs)
    nc = _get_nc(apply_qs, apply_ks)
    res = run_bass_kernel_spmd(nc, in_maps, core_ids=list(range(8)))
    out = np.zeros((B, L, E), np.float32)
    for c in range(8):
        b = c // G
        out[b] += res.results[c]["out"]
    return out


# revision 18
# speedup vs baseline: 1.0642x; 1.0642x over previous
"""Multi-head scaled-cosine attention (B=2, L=2048, E=2048, H=16, D=128) on 8 trn2 cores.

Sharding: core c = (b, g) with b = batch (2), g = head-group of 4 heads (4 groups).
Each core computes its 4 heads' attention for its batch plus the partial output
projection; the host sums the 4 per-group partials per batch.

Precision: matmuls run in bf16 (PSUM accumulation is fp32); the softmax
denominator is accumulated in float32r and reduced across partitions with a
ones-matmul. The Q/K RMS-norm cancels exactly under the subsequent L2
normalization; the L2 reciprocal (and logit scale) are folded into a diagonal
matrix applied by the PE transpose that produces Q^T/K^T. exp(bias - rowmax) is
precomputed on the host and folded in multiplicatively. Scores are built
directly in [k, q] orientation so softmax and attn@V need no on-chip transpose
of the probability matrix. Q/K head dims are host-permuted (evens|odds) so RoPE
uses contiguous vector ops; the permutation cancels inside q.k.
"""
import sys
sys.path.insert(0, '/opt/trn_rl_repo')
import math
import numpy as np
import ml_dtypes

import concourse.bacc as bacc
import concourse.mybir as mybir
import concourse.tile as tile
from concourse.bass_utils import run_bass_kernel_spmd

F32 = mybir.dt.float32
F32R = mybir.dt.float32r
BF16 = mybir.dt.bfloat16
NP_BF16 = ml_dtypes.bfloat16
ALU = mybir.AluOpType
AF = mybir.ActivationFunctionType

B, L, E, H, D = 2, 2048, 2048, 16, 128
G = 4                 # head groups
HPG = H // G          # heads per group = 4
GD = HPG * D          # 512, per-group projection width
P = 128               # partitions
NLT = L // P          # 16 l-tiles
NET = E // P          # 16 e-tiles (contraction)
NQC = L // 512        # 4 q-chunks
NKT = L // P          # 16 k-tiles
HD2 = GD // 2         # 256
LOGIT_SCALE_MAX = math.log(1.0 / 0.01)


def _build(apply_qs: bool, apply_ks: bool):
    nc = bacc.Bacc(None, target_bir_lowering=False)
    d = {}
    d['xqT'] = nc.dram_tensor("xqT", [E, L], BF16, kind="ExternalInput")
    d['xkvT'] = nc.dram_tensor("xkvT", [E, L], BF16, kind="ExternalInput")
    d['expBT'] = nc.dram_tensor("expBT", [L, L], BF16, kind="ExternalInput")
    d['wqT'] = nc.dram_tensor("wqT", [E, GD], BF16, kind="ExternalInput")
    d['wkT'] = nc.dram_tensor("wkT", [E, GD], BF16, kind="ExternalInput")
    d['wvT'] = nc.dram_tensor("wvT", [E, GD], BF16, kind="ExternalInput")
    d['woS'] = nc.dram_tensor("woS", [GD, E], BF16, kind="ExternalInput")
    d['c4q'] = nc.dram_tensor("c4q", [L, HD2], F32, kind="ExternalInput")
    d['s4q'] = nc.dram_tensor("s4q", [L, HD2], F32, kind="ExternalInput")
    d['c4k'] = nc.dram_tensor("c4k", [L, HD2], F32, kind="ExternalInput")
    d['s4k'] = nc.dram_tensor("s4k", [L, HD2], F32, kind="ExternalInput")
    d['ls'] = nc.dram_tensor("ls", [P, HPG], F32, kind="ExternalInput")
    if apply_qs:
        d['qscale'] = nc.dram_tensor("qscale", [P, GD], F32, kind="ExternalInput")
    if apply_ks:
        d['kscale'] = nc.dram_tensor("kscale", [P, GD], F32, kind="ExternalInput")
    out = nc.dram_tensor("out", [L, E], F32, kind="ExternalOutput")

    with tile.TileContext(nc) as tc:
        with tc.tile_pool(name="persist", bufs=1) as persist:
            qT = [persist.tile([P, L], BF16, tag=f"qT{h}", name=f"qT{h}") for h in range(HPG)]
            kT = [persist.tile([P, L], BF16, tag=f"kT{h}", name=f"kT{h}") for h in range(HPG)]
            v_sb = persist.tile([P, NLT, GD], BF16, tag="v_sb")
            identb = persist.tile([P, P], BF16, tag="identb")
            identf = persist.tile([P, P], F32, tag="identf")
            nc.vector.memset(identf[:], 0.0)
            nc.gpsimd.affine_select(out=identf[:], in_=identf[:],
                                    compare_op=ALU.not_equal, fill=1.0, base=0,
                                    pattern=[[-1, P]], channel_multiplier=1)
            nc.vector.tensor_copy(identb[:], identf[:])
            ones_f = persist.tile([P, P], F32, tag="ones_f")
            nc.vector.memset(ones_f[:], 1.0)
            ones_r = persist.tile([P, P], F32R, tag="ones_r")
            nc.scalar.copy(ones_r[:], ones_f[:])
            ones_b = persist.tile([P, P], BF16, tag="ones_b")
            nc.vector.tensor_copy(ones_b[:], ones_f[:])
            ls_t = persist.tile([P, HPG], F32, tag="ls_t")
            nc.sync.dma_start(ls_t[:], d['ls'][:])
            w_all = {}
            for wname in ('wvT', 'wkT', 'wqT'):
                w_all[wname] = persist.tile([P, NET, GD], BF16, tag=wname, name=f"w_{wname}")
                nc.sync.dma_start(
                    w_all[wname][:], d[wname][:].rearrange("(e p) n -> p e n", p=P))
            wo_sb = persist.tile([P, HPG, E], BF16, tag="wo_sb")
            nc.sync.dma_start(
                wo_sb[:], d['woS'][:].rearrange("(h p) e -> p h e", p=P))

            qs_t = ks_t = None
            if apply_qs:
                qs_t = persist.tile([P, GD], F32, tag="qs_t")
                nc.sync.dma_start(qs_t[:], d['qscale'][:])
            if apply_ks:
                ks_t = persist.tile([P, GD], F32, tag="ks_t")
                nc.sync.dma_start(ks_t[:], d['kscale'][:])

            from contextlib import ExitStack
            proj_ctx = ExitStack()
            sbp = proj_ctx.enter_context(tc.tile_pool(name="proj_sb", bufs=3))
            nrm = proj_ctx.enter_context(tc.tile_pool(name="proj_nrm", bufs=3))
            psp = proj_ctx.enter_context(tc.tile_pool(name="proj_ps", bufs=2, space="PSUM"))
            pst = proj_ctx.enter_context(tc.tile_pool(name="proj_pst", bufs=2, space="PSUM"))

            def proj_tile(lt, x_dram, w_sb):
                blk = sbp.tile([P, NET, P], BF16, tag="xblk", name=f"xblk_{lt}")
                nc.sync.dma_start(
                    blk[:],
                    x_dram[:, lt * P:(lt + 1) * P].rearrange("(g p) l -> p g l", p=P))
                psum = psp.tile([P, GD], F32, tag="psum", name=f"psum_{lt}")
                for e in range(NET):
                    nc.tensor.matmul(psum[:], blk[:, e, :], w_sb[:, e, :],
                                     start=(e == 0), stop=(e == NET - 1))
                return psum

            def qk_norm(lt, psum, c_dram, s_dram, scale_tile, use_ls, dstT):
                q1 = nrm.tile([P, GD], BF16, tag="q1")
                nc.scalar.copy(q1[:], psum[:])
                if scale_tile is not None:
                    nc.vector.tensor_mul(q1[:], q1[:], scale_tile[:])
                ct = nrm.tile([P, HD2], F32, tag="ct")
                st = nrm.tile([P, HD2], F32, tag="st")
                nc.sync.dma_start(ct[:], c_dram[lt * P:(lt + 1) * P, :])
                nc.sync.dma_start(st[:], s_dram[lt * P:(lt + 1) * P, :])
                # per-head layout [evens(64) | odds(64)] (host-permuted weights)
                q1v = q1[:].rearrange("p (hh par dd) -> p hh par dd", hh=HPG, par=2)
                qe, qo = q1v[:, :, 0, :], q1v[:, :, 1, :]
                q2 = nrm.tile([P, GD], BF16, tag="q2")
                q2v = q2[:].rearrange("p (hh par dd) -> p hh par dd", hh=HPG, par=2)
                re, ro = q2v[:, :, 0, :], q2v[:, :, 1, :]
                ctv = ct[:].rearrange("p (hh dd) -> p hh dd", hh=HPG)
                stv = st[:].rearrange("p (hh dd) -> p hh dd", hh=HPG)
                tmp = nrm.tile([P, HD2], BF16, tag="tmp")
                tv = tmp[:].rearrange("p (hh dd) -> p hh dd", hh=HPG)
                # evens: qe*c - qo*s ; odds: qo*c + qe*s
                nc.vector.tensor_tensor(tv, qo, stv, ALU.mult)
                nc.vector.tensor_tensor(re, qe, ctv, ALU.mult)
                nc.vector.tensor_sub(re, re, tv)
                nc.vector.tensor_tensor(tv, qe, stv, ALU.mult)
                nc.vector.tensor_tensor(ro, qo, ctv, ALU.mult)
                nc.vector.tensor_add(ro, ro, tv)
                # L2 norm over each head's (now contiguous) D slice
                sqs = nrm.tile([P, GD], BF16, tag="sqs")
                acc = nrm.tile([P, HPG], F32, tag="acc")
                for h in range(HPG):
                    nc.scalar.activation(sqs[:, h * D:(h + 1) * D], q2[:, h * D:(h + 1) * D],
                                         AF.Square, accum_out=acc[:, h:h + 1])
                nrm_t = nrm.tile([P, HPG], F32, tag="nrm_t")
                nc.scalar.activation(nrm_t[:], acc[:], AF.Sqrt)
                nc.vector.tensor_scalar_max(nrm_t[:], nrm_t[:], 1e-12)
                rcp = nrm.tile([P, HPG], F32, tag="rcp")
                nc.vector.reciprocal(rcp[:], nrm_t[:])
                if use_ls:
                    nc.vector.tensor_mul(rcp[:], rcp[:], ls_t[:])
                q3 = nrm.tile([P, GD], BF16, tag="q3")
                for h in range(HPG):
                    nc.vector.tensor_scalar_mul(q3[:, h * D:(h + 1) * D],
                                                q2[:, h * D:(h + 1) * D], rcp[:, h:h + 1])
                for h in range(HPG):
                    pt = pst.tile([P, P], BF16, tag="pt", name=f"pt_{lt}_{h}")
                    nc.tensor.matmul(pt[:], q3[:, h * D:(h + 1) * D], identb[:],
                                     is_transpose=True)
                    nc.any.tensor_copy(dstT[h][:, lt * P:(lt + 1) * P], pt[:])

            # V and K phases over all l-tiles
            for lt in range(NLT):
                psum = proj_tile(lt, d['xkvT'], w_all['wvT'])
                nc.scalar.copy(v_sb[:, lt, :], psum[:])
            for lt in range(NLT):
                psum = proj_tile(lt, d['xkvT'], w_all['wkT'])
                qk_norm(lt, psum, d['c4k'], d['s4k'], ks_t, False, kT)

            for lt in range(NLT):
                psum = proj_tile(lt, d['xqT'], w_all['wqT'])
                qk_norm(lt, psum, d['c4q'], d['s4q'], qs_t, True, qT)
            proj_ctx.close()

            # attention per q-chunk
            att_ctx = ExitStack()
            asb = att_ctx.enter_context(tc.tile_pool(name="att_sb", bufs=3))
            atp = att_ctx.enter_context(tc.tile_pool(name="att_at", bufs=1))
            aop = att_ctx.enter_context(tc.tile_pool(name="att_o", bufs=3))
            ps_pv = att_ctx.enter_context(tc.tile_pool(name="ps_pv", bufs=1, space="PSUM"))
            ps_s = att_ctx.enter_context(tc.tile_pool(name="ps_s", bufs=3, space="PSUM"))
            ps_d = att_ctx.enter_context(tc.tile_pool(name="ps_d", bufs=1, space="PSUM"))
            for qc in range(NQC):
                qsl = slice(qc * 512, (qc + 1) * 512)
                pv = [ps_pv.tile([P, 512], F32, tag=f"pv{h}", name=f"pv{qc}_{h}")
                      for h in range(HPG)]
                den = ps_d.tile([P, 512], F32, tag="den", name=f"den{qc}")
                for kt in range(NKT):
                    eb = asb.tile([P, 512], BF16, tag="eb", name=f"eb{qc}_{kt}")
                    nc.sync.dma_start(eb[:], d['expBT'][kt * P:(kt + 1) * P, qsl])
                    p_t = asb.tile([P, 2048], BF16, tag="p_t", name=f"pt{qc}_{kt}", bufs=3)
                    ebb = eb[:].rearrange("p (o q) -> p o q", o=1).broadcast_to([P, 2, 512])
                    for hp in range(2):
                        for i in range(2):
                            h = 2 * hp + i
                            s_ps = ps_s.tile([P, 512], F32, tag="s_ps", name=f"sp{qc}_{kt}_{h}")
                            nc.tensor.matmul(s_ps[:], kT[h][:, kt * P:(kt + 1) * P],
                                             qT[h][:, qsl], start=True, stop=True)
                            nc.scalar.activation(p_t[:, h * 512:(h + 1) * 512], s_ps[:], AF.Exp)
                        half = p_t[:, hp * 1024:(hp + 1) * 1024].rearrange("p (i q) -> p i q", i=2)
                        nc.vector.tensor_tensor(half, half, ebb, ALU.mult)
                        for i in range(2):
                            h = 2 * hp + i
                            nc.tensor.matmul(pv[h][:], v_sb[:, kt, h * D:(h + 1) * D],
                                             p_t[:, h * 512:(h + 1) * 512],
                                             start=(kt == 0), stop=(kt == NKT - 1))
                        for i in range(2):
                            h = 2 * hp + i
                            nc.tensor.matmul(den[32 * h:32 * h + 32, :],
                                             ones_b[:, 0:32],
                                             p_t[:, h * 512:(h + 1) * 512],
                                             start=(kt == 0), stop=(kt == NKT - 1),
                                             tile_position=(0, 32 * h))
                attn = [atp.tile([P, 512], BF16, tag=f"at{h}", name=f"at{qc}_{h}")
                        for h in range(HPG)]
                rcp32s = []
                for i in range(2):
                    lnd = asb.tile([64, 512], F32, tag=f"lnd{i}", name=f"lnd{qc}_{i}")
                    nc.scalar.activation(lnd[:], den[64 * i:64 * i + 64, :], AF.Ln)
                    rcp32 = asb.tile([64, 512], F32R, tag=f"rcp32{i}", name=f"rcp32{qc}_{i}")
                    nc.scalar.activation(rcp32[:], lnd[:], AF.Exp, scale=-1.0)
                    rcp32s.append(rcp32)
                for h in range(HPG):
                    b_ps = ps_s.tile([P, 512], F32, tag="s_ps", name=f"b{qc}_{h}")
                    r32 = rcp32s[h // 2]
                    off = 32 * (h % 2)
                    nc.tensor.matmul(b_ps[:], ones_r[off:off + 1, :],
                                     r32[off:off + 1, :], start=True, stop=True)
                    rcpb = asb.tile([P, 512], BF16, tag="rcpb", name=f"rb{qc}_{h}")
                    nc.vector.tensor_copy(rcpb[:], b_ps[:])
                    nc.vector.tensor_mul(attn[h][:], pv[h][:], rcpb[:])
                for lsub in range(4):
                    for ec in range(4):
                        o_ps = ps_s.tile([P, 512], F32, tag="s_ps", name=f"o{qc}_{lsub}_{ec}")
                        for h in range(HPG):
                            nc.tensor.matmul(o_ps[:], attn[h][:, lsub * P:(lsub + 1) * P],
                                             wo_sb[:, h, ec * 512:(ec + 1) * 512],
                                             start=(h == 0), stop=(h == HPG - 1))
                        o_sb = aop.tile([P, 512], F32, tag="o_sb", name=f"ob{qc}_{lsub}_{ec}")
                        nc.vector.tensor_copy(o_sb[:], o_ps[:])
                        nc.sync.dma_start(
                            out[qc * 512 + lsub * P: qc * 512 + (lsub + 1) * P,
                                ec * 512:(ec + 1) * 512], o_sb[:])
            att_ctx.close()
    nc.compile()
    return nc


# head-dim permutation: within each head, evens first then odds
_PERM = np.empty(GD, np.int64)
for _i in range(GD):
    _h, _j = divmod(_i, D)
    _par, _dd = divmod(_j, D // 2)
    _PERM[_i] = _h * D + 2 * _dd + _par


def _prepare(inputs):
    f32 = np.float32
    inputs_q = np.asarray(inputs["inputs_q"], f32)
    inputs_kv = np.asarray(inputs["inputs_kv"], f32)
    bias = np.asarray(inputs["bias"], f32).reshape(L, L)
    q_sin = np.asarray(inputs["q_sinusoids"], f32)
    k_sin = np.asarray(inputs["k_sinusoids"], f32)
    Wq = np.asarray(inputs["Wq"], f32)
    Wk = np.asarray(inputs["Wk"], f32)
    Wv = np.asarray(inputs["Wv"], f32)
    Wo = np.asarray(inputs["Wo"], f32)
    qns = np.asarray(inputs["q_norm_scale"], f32)
    kns = np.asarray(inputs["k_norm_scale"], f32)
    ls = np.asarray(inputs["logit_scale"], f32)

    apply_qs = not np.all(qns == 1.0)
    apply_ks = not np.all(kns == 1.0)

    bm = bias.max(axis=1, keepdims=True)
    expBT = np.ascontiguousarray(np.exp((bias - bm).T).astype(NP_BF16))
    ls_e = np.exp(np.minimum(ls, LOGIT_SCALE_MAX)).astype(f32)

    per_b = []
    for b in range(B):
        per_b.append(dict(
            xqT=np.ascontiguousarray(inputs_q[b].T.astype(NP_BF16)),
            xkvT=np.ascontiguousarray(inputs_kv[b].T.astype(NP_BF16)),
            c4q=np.ascontiguousarray(np.tile(q_sin[b][:, 0::2], (1, HPG))),
            s4q=np.ascontiguousarray(np.tile(q_sin[b][:, 1::2], (1, HPG))),
            c4k=np.ascontiguousarray(np.tile(k_sin[b][:, 0::2], (1, HPG))),
            s4k=np.ascontiguousarray(np.tile(k_sin[b][:, 1::2], (1, HPG))),
        ))
    per_g = []
    for g in range(G):
        rows = slice(g * GD, (g + 1) * GD)
        per_g.append(dict(
            wqT=np.ascontiguousarray(Wq[rows, :][_PERM, :].T.astype(NP_BF16)),
            wkT=np.ascontiguousarray(Wk[rows, :][_PERM, :].T.astype(NP_BF16)),
            wvT=np.ascontiguousarray(Wv[rows, :].T.astype(NP_BF16)),
            woS=np.ascontiguousarray(Wo[:, rows].T.astype(NP_BF16)),
            ls=np.broadcast_to(ls_e[g * HPG:(g + 1) * HPG][None, :], (P, HPG)).copy(),
        ))

    qs_bc = (np.broadcast_to(np.tile(qns, HPG)[_PERM][None, :], (P, GD)).copy()
             if apply_qs else None)
    ks_bc = (np.broadcast_to(np.tile(kns, HPG)[_PERM][None, :], (P, GD)).copy()
             if apply_ks else None)

    in_maps = []
    for c in range(8):
        b, g = divmod(c, G)
        m = dict(expBT=expBT)
        m.update(per_b[b])
        m.update(per_g[g])
        if apply_qs:
            m['qscale'] = qs_bc
        if apply_ks:
            m['kscale'] = ks_bc
        in_maps.append(m)
    return in_maps, apply_qs, apply_ks


_CACHE = {}


def _get_nc(apply_qs, apply_ks):
    key = (apply_qs, apply_ks)
    if key not in _CACHE:
        _CACHE[key] = _build(apply_qs, apply_ks)
    return _CACHE[key]


def kernel(**inputs) -> np.ndarray:
    in_maps, apply_qs, apply_ks = _prepare(inputs)
    nc = _get_nc(apply_qs, apply_ks)
    res = run_bass_kernel_spmd(nc, in_maps, core_ids=list(range(8)))
    out = np.zeros((B, L, E), np.float32)
    for c in range(8):
        b = c // G
        out[b] += res.results[c]["out"]
    return out


# revision 19
# speedup vs baseline: 1.0974x; 1.0312x over previous
"""Multi-head scaled-cosine attention (B=2, L=2048, E=2048, H=16, D=128) on 8 trn2 cores.

Sharding: core c = (b, g) with b = batch (2), g = head-group of 4 heads (4 groups).
Each core computes its 4 heads' attention for its batch plus the partial output
projection; the host sums the 4 per-group partials per batch.

Precision: matmuls run in bf16 (PSUM accumulation is fp32); the softmax
denominator is accumulated in float32r and reduced across partitions with a
ones-matmul. The Q/K RMS-norm cancels exactly under the subsequent L2
normalization; the L2 reciprocal (and logit scale) are folded into a diagonal
matrix applied by the PE transpose that produces Q^T/K^T. exp(bias - rowmax) is
precomputed on the host and folded in multiplicatively. Scores are built
directly in [k, q] orientation so softmax and attn@V need no on-chip transpose
of the probability matrix. Q/K head dims are host-permuted (evens|odds) so RoPE
uses contiguous vector ops; the permutation cancels inside q.k.
"""
import sys
sys.path.insert(0, '/opt/trn_rl_repo')
import math
import numpy as np
import ml_dtypes

import concourse.bacc as bacc
import concourse.mybir as mybir
import concourse.tile as tile
from concourse.bass_utils import run_bass_kernel_spmd

F32 = mybir.dt.float32
F32R = mybir.dt.float32r
BF16 = mybir.dt.bfloat16
NP_BF16 = ml_dtypes.bfloat16
ALU = mybir.AluOpType
AF = mybir.ActivationFunctionType

B, L, E, H, D = 2, 2048, 2048, 16, 128
G = 4                 # head groups
HPG = H // G          # heads per group = 4
GD = HPG * D          # 512, per-group projection width
P = 128               # partitions
NLT = L // P          # 16 l-tiles
NET = E // P          # 16 e-tiles (contraction)
NQC = L // 512        # 4 q-chunks
NKT = L // P          # 16 k-tiles
HD2 = GD // 2         # 256
LOGIT_SCALE_MAX = math.log(1.0 / 0.01)


def _build(apply_qs: bool, apply_ks: bool):
    nc = bacc.Bacc(None, target_bir_lowering=False)
    d = {}
    d['xqT'] = nc.dram_tensor("xqT", [E, L], BF16, kind="ExternalInput")
    d['xkvT'] = nc.dram_tensor("xkvT", [E, L], BF16, kind="ExternalInput")
    d['expBT'] = nc.dram_tensor("expBT", [L, L], BF16, kind="ExternalInput")
    d['wqT'] = nc.dram_tensor("wqT", [E, GD], BF16, kind="ExternalInput")
    d['wkT'] = nc.dram_tensor("wkT", [E, GD], BF16, kind="ExternalInput")
    d['wvT'] = nc.dram_tensor("wvT", [E, GD], BF16, kind="ExternalInput")
    d['woS'] = nc.dram_tensor("woS", [GD, E], BF16, kind="ExternalInput")
    d['c4q'] = nc.dram_tensor("c4q", [L, HD2], F32, kind="ExternalInput")
    d['s4q'] = nc.dram_tensor("s4q", [L, HD2], F32, kind="ExternalInput")
    d['c4k'] = nc.dram_tensor("c4k", [L, HD2], F32, kind="ExternalInput")
    d['s4k'] = nc.dram_tensor("s4k", [L, HD2], F32, kind="ExternalInput")
    d['ls'] = nc.dram_tensor("ls", [P, HPG], F32, kind="ExternalInput")
    if apply_qs:
        d['qscale'] = nc.dram_tensor("qscale", [P, GD], F32, kind="ExternalInput")
    if apply_ks:
        d['kscale'] = nc.dram_tensor("kscale", [P, GD], F32, kind="ExternalInput")
    out = nc.dram_tensor("out", [L, E], F32, kind="ExternalOutput")

    with tile.TileContext(nc) as tc:
        with tc.tile_pool(name="persist", bufs=1) as persist:
            qT = [persist.tile([P, L], BF16, tag=f"qT{h}", name=f"qT{h}") for h in range(HPG)]
            kT = [persist.tile([P, L], BF16, tag=f"kT{h}", name=f"kT{h}") for h in range(HPG)]
            v_sb = persist.tile([P, NLT, GD], BF16, tag="v_sb")
            identb = persist.tile([P, P], BF16, tag="identb")
            identf = persist.tile([P, P], F32, tag="identf")
            nc.vector.memset(identf[:], 0.0)
            nc.gpsimd.affine_select(out=identf[:], in_=identf[:],
                                    compare_op=ALU.not_equal, fill=1.0, base=0,
                                    pattern=[[-1, P]], channel_multiplier=1)
            nc.vector.tensor_copy(identb[:], identf[:])
            ones_f = persist.tile([P, P], F32, tag="ones_f")
            nc.vector.memset(ones_f[:], 1.0)
            ones_r = persist.tile([P, P], F32R, tag="ones_r")
            nc.scalar.copy(ones_r[:], ones_f[:])
            ones_b = persist.tile([P, P], BF16, tag="ones_b")
            nc.vector.tensor_copy(ones_b[:], ones_f[:])
            ls_t = persist.tile([P, HPG], F32, tag="ls_t")
            nc.sync.dma_start(ls_t[:], d['ls'][:])
            w_all = {}
            for wname in ('wvT', 'wkT', 'wqT'):
                w_all[wname] = persist.tile([P, NET, GD], BF16, tag=wname, name=f"w_{wname}")
                nc.sync.dma_start(
                    w_all[wname][:], d[wname][:].rearrange("(e p) n -> p e n", p=P))
            wo_sb = persist.tile([P, HPG, E], BF16, tag="wo_sb")
            nc.sync.dma_start(
                wo_sb[:], d['woS'][:].rearrange("(h p) e -> p h e", p=P))

            qs_t = ks_t = None
            if apply_qs:
                qs_t = persist.tile([P, GD], F32, tag="qs_t")
                nc.sync.dma_start(qs_t[:], d['qscale'][:])
            if apply_ks:
                ks_t = persist.tile([P, GD], F32, tag="ks_t")
                nc.sync.dma_start(ks_t[:], d['kscale'][:])

            from contextlib import ExitStack
            proj_ctx = ExitStack()
            sbp = proj_ctx.enter_context(tc.tile_pool(name="proj_sb", bufs=3))
            nrm = proj_ctx.enter_context(tc.tile_pool(name="proj_nrm", bufs=3))
            psp = proj_ctx.enter_context(tc.tile_pool(name="proj_ps", bufs=2, space="PSUM"))
            pst = proj_ctx.enter_context(tc.tile_pool(name="proj_pst", bufs=2, space="PSUM"))

            def proj_tile(lt, x_dram, w_sb):
                blk = sbp.tile([P, NET, P], BF16, tag="xblk", name=f"xblk_{lt}")
                nc.sync.dma_start(
                    blk[:],
                    x_dram[:, lt * P:(lt + 1) * P].rearrange("(g p) l -> p g l", p=P))
                psum = psp.tile([P, GD], F32, tag="psum", name=f"psum_{lt}")
                for e in range(NET):
                    nc.tensor.matmul(psum[:], blk[:, e, :], w_sb[:, e, :],
                                     start=(e == 0), stop=(e == NET - 1))
                return psum

            def qk_norm(lt, psum, c_dram, s_dram, scale_tile, use_ls, dstT):
                q1 = nrm.tile([P, GD], BF16, tag="q1")
                nc.scalar.copy(q1[:], psum[:])
                if scale_tile is not None:
                    nc.vector.tensor_mul(q1[:], q1[:], scale_tile[:])
                ct = nrm.tile([P, HD2], F32, tag="ct")
                st = nrm.tile([P, HD2], F32, tag="st")
                nc.sync.dma_start(ct[:], c_dram[lt * P:(lt + 1) * P, :])
                nc.sync.dma_start(st[:], s_dram[lt * P:(lt + 1) * P, :])
                # per-head layout [evens(64) | odds(64)] (host-permuted weights)
                q1v = q1[:].rearrange("p (hh par dd) -> p hh par dd", hh=HPG, par=2)
                qe, qo = q1v[:, :, 0, :], q1v[:, :, 1, :]
                q2 = nrm.tile([P, GD], BF16, tag="q2")
                q2v = q2[:].rearrange("p (hh par dd) -> p hh par dd", hh=HPG, par=2)
                re, ro = q2v[:, :, 0, :], q2v[:, :, 1, :]
                ctv = ct[:].rearrange("p (hh dd) -> p hh dd", hh=HPG)
                stv = st[:].rearrange("p (hh dd) -> p hh dd", hh=HPG)
                tmp = nrm.tile([P, HD2], BF16, tag="tmp")
                tv = tmp[:].rearrange("p (hh dd) -> p hh dd", hh=HPG)
                # evens: qe*c - qo*s ; odds: qo*c + qe*s
                nc.vector.tensor_tensor(tv, qo, stv, ALU.mult)
                nc.vector.tensor_tensor(re, qe, ctv, ALU.mult)
                nc.vector.tensor_sub(re, re, tv)
                nc.vector.tensor_tensor(tv, qe, stv, ALU.mult)
                nc.vector.tensor_tensor(ro, qo, ctv, ALU.mult)
                nc.vector.tensor_add(ro, ro, tv)
                # L2 norm over each head's (now contiguous) D slice
                sqs = nrm.tile([P, GD], BF16, tag="sqs")
                acc = nrm.tile([P, HPG], F32, tag="acc")
                for h in range(HPG):
                    nc.scalar.activation(sqs[:, h * D:(h + 1) * D], q2[:, h * D:(h + 1) * D],
                                         AF.Square, accum_out=acc[:, h:h + 1])
                nrm_t = nrm.tile([P, HPG], F32, tag="nrm_t")
                nc.scalar.activation(nrm_t[:], acc[:], AF.Sqrt)
                nc.vector.tensor_scalar_max(nrm_t[:], nrm_t[:], 1e-12)
                rcp = nrm.tile([P, HPG], F32, tag="rcp")
                nc.vector.reciprocal(rcp[:], nrm_t[:])
                if use_ls:
                    nc.vector.tensor_mul(rcp[:], rcp[:], ls_t[:])
                q3 = nrm.tile([P, GD], BF16, tag="q3")
                for h in range(HPG):
                    nc.vector.tensor_scalar_mul(q3[:, h * D:(h + 1) * D],
                                                q2[:, h * D:(h + 1) * D], rcp[:, h:h + 1])
                for h in range(HPG):
                    pt = pst.tile([P, P], BF16, tag="pt", name=f"pt_{lt}_{h}")
                    nc.tensor.matmul(pt[:], q3[:, h * D:(h + 1) * D], identb[:],
                                     is_transpose=True)
                    nc.any.tensor_copy(dstT[h][:, lt * P:(lt + 1) * P], pt[:])

            # V and K phases over all l-tiles
            for lt in range(NLT):
                psum = proj_tile(lt, d['xkvT'], w_all['wvT'])
                nc.scalar.copy(v_sb[:, lt, :], psum[:])
            for lt in range(NLT):
                psum = proj_tile(lt, d['xkvT'], w_all['wkT'])
                qk_norm(lt, psum, d['c4k'], d['s4k'], ks_t, False, kT)

            for lt in range(NLT):
                psum = proj_tile(lt, d['xqT'], w_all['wqT'])
                qk_norm(lt, psum, d['c4q'], d['s4q'], qs_t, True, qT)
            proj_ctx.close()

            # attention per q-chunk
            att_ctx = ExitStack()
            asb = att_ctx.enter_context(tc.tile_pool(name="att_sb", bufs=3))
            atp = att_ctx.enter_context(tc.tile_pool(name="att_at", bufs=1))
            aop = att_ctx.enter_context(tc.tile_pool(name="att_o", bufs=3))
            ps_pv = att_ctx.enter_context(tc.tile_pool(name="ps_pv", bufs=1, space="PSUM"))
            ps_s = att_ctx.enter_context(tc.tile_pool(name="ps_s", bufs=3, space="PSUM"))
            ps_d = att_ctx.enter_context(tc.tile_pool(name="ps_d", bufs=1, space="PSUM"))
            for qc in range(NQC):
                qsl = slice(qc * 512, (qc + 1) * 512)
                pv = [ps_pv.tile([P, 512], F32, tag=f"pv{h}", name=f"pv{qc}_{h}")
                      for h in range(HPG)]
                den = ps_d.tile([P, 512], F32, tag="den", name=f"den{qc}")
                def stage1(kt):
                    eb = asb.tile([P, 512], BF16, tag="eb", name=f"eb{qc}_{kt}")
                    nc.sync.dma_start(eb[:], d['expBT'][kt * P:(kt + 1) * P, qsl])
                    p_t = asb.tile([P, 2048], BF16, tag="p_t", name=f"pt{qc}_{kt}", bufs=3)
                    ebb = eb[:].rearrange("p (o q) -> p o q", o=1).broadcast_to([P, 2, 512])
                    for hp in range(2):
                        for i in range(2):
                            h = 2 * hp + i
                            s_ps = ps_s.tile([P, 512], F32, tag="s_ps", name=f"sp{qc}_{kt}_{h}")
                            nc.tensor.matmul(s_ps[:], kT[h][:, kt * P:(kt + 1) * P],
                                             qT[h][:, qsl], start=True, stop=True)
                            nc.scalar.activation(p_t[:, h * 512:(h + 1) * 512], s_ps[:], AF.Exp)
                        half = p_t[:, hp * 1024:(hp + 1) * 1024].rearrange("p (i q) -> p i q", i=2)
                        nc.vector.tensor_tensor(half, half, ebb, ALU.mult)
                    return p_t

                def stage2(kt, p_t):
                    for h in range(HPG):
                        nc.tensor.matmul(pv[h][:], v_sb[:, kt, h * D:(h + 1) * D],
                                         p_t[:, h * 512:(h + 1) * 512],
                                         start=(kt == 0), stop=(kt == NKT - 1))
                    for h in range(HPG):
                        nc.tensor.matmul(den[32 * h:32 * h + 32, :],
                                         ones_b[:, 0:32],
                                         p_t[:, h * 512:(h + 1) * 512],
                                         start=(kt == 0), stop=(kt == NKT - 1),
                                         tile_position=(0, 32 * h))

                prev = None
                for kt in range(NKT):
                    p_t = stage1(kt)
                    if prev is not None:
                        stage2(prev[0], prev[1])
                    prev = (kt, p_t)
                stage2(prev[0], prev[1])
                attn = [atp.tile([P, 512], BF16, tag=f"at{h}", name=f"at{qc}_{h}")
                        for h in range(HPG)]
                rcp32s = []
                for i in range(2):
                    lnd = asb.tile([64, 512], F32, tag=f"lnd{i}", name=f"lnd{qc}_{i}")
                    nc.scalar.activation(lnd[:], den[64 * i:64 * i + 64, :], AF.Ln)
                    rcp32 = asb.tile([64, 512], F32R, tag=f"rcp32{i}", name=f"rcp32{qc}_{i}")
                    nc.scalar.activation(rcp32[:], lnd[:], AF.Exp, scale=-1.0)
                    rcp32s.append(rcp32)
                for h in range(HPG):
                    b_ps = ps_s.tile([P, 512], F32, tag="s_ps", name=f"b{qc}_{h}")
                    r32 = rcp32s[h // 2]
                    off = 32 * (h % 2)
                    nc.tensor.matmul(b_ps[:], ones_r[off:off + 1, :],
                                     r32[off:off + 1, :], start=True, stop=True)
                    rcpb = asb.tile([P, 512], BF16, tag="rcpb", name=f"rb{qc}_{h}")
                    nc.vector.tensor_copy(rcpb[:], b_ps[:])
                    nc.vector.tensor_mul(attn[h][:], pv[h][:], rcpb[:])
                for lsub in range(4):
                    for ec in range(4):
                        o_ps = ps_s.tile([P, 512], F32, tag="s_ps", name=f"o{qc}_{lsub}_{ec}")
                        for h in range(HPG):
                            nc.tensor.matmul(o_ps[:], attn[h][:, lsub * P:(lsub + 1) * P],
                                             wo_sb[:, h, ec * 512:(ec + 1) * 512],
                                             start=(h == 0), stop=(h == HPG - 1))
                        o_sb = aop.tile([P, 512], F32, tag="o_sb", name=f"ob{qc}_{lsub}_{ec}")
                        nc.vector.tensor_copy(o_sb[:], o_ps[:])
                        nc.sync.dma_start(
                            out[qc * 512 + lsub * P: qc * 512 + (lsub + 1) * P,
                                ec * 512:(ec + 1) * 512], o_sb[:])
            att_ctx.close()
    nc.compile()
    return nc


# head-dim permutation: within each head, evens first then odds
_PERM = np.empty(GD, np.int64)
for _i in range(GD):
    _h, _j = divmod(_i, D)
    _par, _dd = divmod(_j, D // 2)
    _PERM[_i] = _h * D + 2 * _dd + _par


def _prepare(inputs):
    f32 = np.float32
    inputs_q = np.asarray(inputs["inputs_q"], f32)
    inputs_kv = np.asarray(inputs["inputs_kv"], f32)
    bias = np.asarray(inputs["bias"], f32).reshape(L, L)
    q_sin = np.asarray(inputs["q_sinusoids"], f32)
    k_sin = np.asarray(inputs["k_sinusoids"], f32)
    Wq = np.asarray(inputs["Wq"], f32)
    Wk = np.asarray(inputs["Wk"], f32)
    Wv = np.asarray(inputs["Wv"], f32)
    Wo = np.asarray(inputs["Wo"], f32)
    qns = np.asarray(inputs["q_norm_scale"], f32)
    kns = np.asarray(inputs["k_norm_scale"], f32)
    ls = np.asarray(inputs["logit_scale"], f32)

    apply_qs = not np.all(qns == 1.0)
    apply_ks = not np.all(kns == 1.0)

    bm = bias.max(axis=1, keepdims=True)
    expBT = np.ascontiguousarray(np.exp((bias - bm).T).astype(NP_BF16))
    ls_e = np.exp(np.minimum(ls, LOGIT_SCALE_MAX)).astype(f32)

    per_b = []
    for b in range(B):
        per_b.append(dict(
            xqT=np.ascontiguousarray(inputs_q[b].T.astype(NP_BF16)),
            xkvT=np.ascontiguousarray(inputs_kv[b].T.astype(NP_BF16)),
            c4q=np.ascontiguousarray(np.tile(q_sin[b][:, 0::2], (1, HPG))),
            s4q=np.ascontiguousarray(np.tile(q_sin[b][:, 1::2], (1, HPG))),
            c4k=np.ascontiguousarray(np.tile(k_sin[b][:, 0::2], (1, HPG))),
            s4k=np.ascontiguousarray(np.tile(k_sin[b][:, 1::2], (1, HPG))),
        ))
    per_g = []
    for g in range(G):
        rows = slice(g * GD, (g + 1) * GD)
        per_g.append(dict(
            wqT=np.ascontiguousarray(Wq[rows, :][_PERM, :].T.astype(NP_BF16)),
            wkT=np.ascontiguousarray(Wk[rows, :][_PERM, :].T.astype(NP_BF16)),
            wvT=np.ascontiguousarray(Wv[rows, :].T.astype(NP_BF16)),
            woS=np.ascontiguousarray(Wo[:, rows].T.astype(NP_BF16)),
            ls=np.broadcast_to(ls_e[g * HPG:(g + 1) * HPG][None, :], (P, HPG)).copy(),
        ))

    qs_bc = (np.broadcast_to(np.tile(qns, HPG)[_PERM][None, :], (P, GD)).copy()
             if apply_qs else None)
    ks_bc = (np.broadcast_to(np.tile(kns, HPG)[_PERM][None, :], (P, GD)).copy()
             if apply_ks else None)

    in_maps = []
    for c in range(8):
        b, g = divmod(c, G)
        m = dict(expBT=expBT)
        m.update(per_b[b])
        m.update(per_g[g])
        if apply_qs:
            m['qscale'] = qs_bc
        if apply_ks:
            m['kscale'] = ks_bc
        in_maps.append(m)
    return in_maps, apply_qs, apply_ks


_CACHE = {}


def _get_nc(apply_qs, apply_ks):
    key = (apply_qs, apply_ks)
    if key not in _CACHE:
        _CACHE[key] = _build(apply_qs, apply_ks)
    return _CACHE[key]


def kernel(**inputs) -> np.ndarray:
    in_maps, apply_qs, apply_ks = _prepare(inputs)
    nc = _get_nc(apply_qs, apply_ks)
    res = run_bass_kernel_spmd(nc, in_maps, core_ids=list(range(8)))
    out = np.zeros((B, L, E), np.float32)
    for c in range(8):
        b = c // G
        out[b] += res.results[c]["out"]
    return out


# revision 20
# speedup vs baseline: 1.1004x; 1.0028x over previous
"""Multi-head scaled-cosine attention (B=2, L=2048, E=2048, H=16, D=128) on 8 trn2 cores.

Sharding: core c = (b, g) with b = batch (2), g = head-group of 4 heads (4 groups).
Each core computes its 4 heads' attention for its batch plus the partial output
projection; the host sums the 4 per-group partials per batch.

Precision: matmuls run in bf16 (PSUM accumulation is fp32); the softmax
denominator is accumulated in float32r and reduced across partitions with a
ones-matmul. The Q/K RMS-norm cancels exactly under the subsequent L2
normalization; the L2 reciprocal (and logit scale) are folded into a diagonal
matrix applied by the PE transpose that produces Q^T/K^T. exp(bias - rowmax) is
precomputed on the host and folded in multiplicatively. Scores are built
directly in [k, q] orientation so softmax and attn@V need no on-chip transpose
of the probability matrix. Q/K head dims are host-permuted (evens|odds) so RoPE
uses contiguous vector ops; the permutation cancels inside q.k.
"""
import sys
sys.path.insert(0, '/opt/trn_rl_repo')
import math
import numpy as np
import ml_dtypes

import concourse.bacc as bacc
import concourse.mybir as mybir
import concourse.tile as tile
from concourse.bass_utils import run_bass_kernel_spmd

F32 = mybir.dt.float32
F32R = mybir.dt.float32r
BF16 = mybir.dt.bfloat16
NP_BF16 = ml_dtypes.bfloat16
ALU = mybir.AluOpType
AF = mybir.ActivationFunctionType

B, L, E, H, D = 2, 2048, 2048, 16, 128
G = 4                 # head groups
HPG = H // G          # heads per group = 4
GD = HPG * D          # 512, per-group projection width
P = 128               # partitions
NLT = L // P          # 16 l-tiles
NET = E // P          # 16 e-tiles (contraction)
NQC = L // 512        # 4 q-chunks
NKT = L // P          # 16 k-tiles
HD2 = GD // 2         # 256
LOGIT_SCALE_MAX = math.log(1.0 / 0.01)


def _build(apply_qs: bool, apply_ks: bool):
    nc = bacc.Bacc(None, target_bir_lowering=False)
    d = {}
    d['xqT'] = nc.dram_tensor("xqT", [E, L], BF16, kind="ExternalInput")
    d['xkvT'] = nc.dram_tensor("xkvT", [E, L], BF16, kind="ExternalInput")
    d['expBT'] = nc.dram_tensor("expBT", [L, L], BF16, kind="ExternalInput")
    d['wqT'] = nc.dram_tensor("wqT", [E, GD], BF16, kind="ExternalInput")
    d['wkT'] = nc.dram_tensor("wkT", [E, GD], BF16, kind="ExternalInput")
    d['wvT'] = nc.dram_tensor("wvT", [E, GD], BF16, kind="ExternalInput")
    d['woS'] = nc.dram_tensor("woS", [GD, E], BF16, kind="ExternalInput")
    d['c4q'] = nc.dram_tensor("c4q", [L, HD2], F32, kind="ExternalInput")
    d['s4q'] = nc.dram_tensor("s4q", [L, HD2], F32, kind="ExternalInput")
    d['c4k'] = nc.dram_tensor("c4k", [L, HD2], F32, kind="ExternalInput")
    d['s4k'] = nc.dram_tensor("s4k", [L, HD2], F32, kind="ExternalInput")
    d['ls'] = nc.dram_tensor("ls", [P, HPG], F32, kind="ExternalInput")
    if apply_qs:
        d['qscale'] = nc.dram_tensor("qscale", [P, GD], F32, kind="ExternalInput")
    if apply_ks:
        d['kscale'] = nc.dram_tensor("kscale", [P, GD], F32, kind="ExternalInput")
    out = nc.dram_tensor("out", [L, E], F32, kind="ExternalOutput")

    with tile.TileContext(nc) as tc:
        with tc.tile_pool(name="persist", bufs=1) as persist:
            qT = [persist.tile([P, L], BF16, tag=f"qT{h}", name=f"qT{h}") for h in range(HPG)]
            kT = [persist.tile([P, L], BF16, tag=f"kT{h}", name=f"kT{h}") for h in range(HPG)]
            v_sb = persist.tile([P, NLT, GD], BF16, tag="v_sb")
            identb = persist.tile([P, P], BF16, tag="identb")
            identf = persist.tile([P, P], F32, tag="identf")
            nc.vector.memset(identf[:], 0.0)
            nc.gpsimd.affine_select(out=identf[:], in_=identf[:],
                                    compare_op=ALU.not_equal, fill=1.0, base=0,
                                    pattern=[[-1, P]], channel_multiplier=1)
            nc.vector.tensor_copy(identb[:], identf[:])
            ones_f = persist.tile([P, P], F32, tag="ones_f")
            nc.vector.memset(ones_f[:], 1.0)
            ones_r = persist.tile([P, P], F32R, tag="ones_r")
            nc.scalar.copy(ones_r[:], ones_f[:])
            ones_b = persist.tile([P, P], BF16, tag="ones_b")
            nc.vector.tensor_copy(ones_b[:], ones_f[:])
            ls_t = persist.tile([P, HPG], F32, tag="ls_t")
            nc.sync.dma_start(ls_t[:], d['ls'][:])
            w_all = {}
            for wname in ('wvT', 'wkT', 'wqT'):
                w_all[wname] = persist.tile([P, NET, GD], BF16, tag=wname, name=f"w_{wname}")
                nc.sync.dma_start(
                    w_all[wname][:], d[wname][:].rearrange("(e p) n -> p e n", p=P))
            wo_sb = persist.tile([P, HPG, E], BF16, tag="wo_sb")
            nc.sync.dma_start(
                wo_sb[:], d['woS'][:].rearrange("(h p) e -> p h e", p=P))

            qs_t = ks_t = None
            if apply_qs:
                qs_t = persist.tile([P, GD], F32, tag="qs_t")
                nc.sync.dma_start(qs_t[:], d['qscale'][:])
            if apply_ks:
                ks_t = persist.tile([P, GD], F32, tag="ks_t")
                nc.sync.dma_start(ks_t[:], d['kscale'][:])

            from contextlib import ExitStack
            proj_ctx = ExitStack()
            sbp = proj_ctx.enter_context(tc.tile_pool(name="proj_sb", bufs=3))
            nrm = proj_ctx.enter_context(tc.tile_pool(name="proj_nrm", bufs=4))
            psp = proj_ctx.enter_context(tc.tile_pool(name="proj_ps", bufs=2, space="PSUM"))
            pst = proj_ctx.enter_context(tc.tile_pool(name="proj_pst", bufs=2, space="PSUM"))

            def proj_tile(lt, x_dram, w_sb):
                blk = sbp.tile([P, NET, P], BF16, tag="xblk", name=f"xblk_{lt}")
                nc.sync.dma_start(
                    blk[:],
                    x_dram[:, lt * P:(lt + 1) * P].rearrange("(g p) l -> p g l", p=P))
                psum = psp.tile([P, GD], F32, tag="psum", name=f"psum_{lt}")
                for e in range(NET):
                    nc.tensor.matmul(psum[:], blk[:, e, :], w_sb[:, e, :],
                                     start=(e == 0), stop=(e == NET - 1))
                return psum

            def qk_norm(lt, psum, c_dram, s_dram, scale_tile, use_ls, dstT):
                q1 = nrm.tile([P, GD], BF16, tag="q1")
                nc.scalar.copy(q1[:], psum[:])
                if scale_tile is not None:
                    nc.vector.tensor_mul(q1[:], q1[:], scale_tile[:])
                ct = nrm.tile([P, HD2], F32, tag="ct")
                st = nrm.tile([P, HD2], F32, tag="st")
                nc.sync.dma_start(ct[:], c_dram[lt * P:(lt + 1) * P, :])
                nc.sync.dma_start(st[:], s_dram[lt * P:(lt + 1) * P, :])
                # per-head layout [evens(64) | odds(64)] (host-permuted weights)
                q1v = q1[:].rearrange("p (hh par dd) -> p hh par dd", hh=HPG, par=2)
                qe, qo = q1v[:, :, 0, :], q1v[:, :, 1, :]
                q2 = nrm.tile([P, GD], BF16, tag="q2")
                q2v = q2[:].rearrange("p (hh par dd) -> p hh par dd", hh=HPG, par=2)
                re, ro = q2v[:, :, 0, :], q2v[:, :, 1, :]
                ctv = ct[:].rearrange("p (hh dd) -> p hh dd", hh=HPG)
                stv = st[:].rearrange("p (hh dd) -> p hh dd", hh=HPG)
                tmp = nrm.tile([P, HD2], BF16, tag="tmp")
                tv = tmp[:].rearrange("p (hh dd) -> p hh dd", hh=HPG)
                # evens: qe*c - qo*s ; odds: qo*c + qe*s
                nc.vector.tensor_tensor(tv, qo, stv, ALU.mult)
                nc.vector.tensor_tensor(re, qe, ctv, ALU.mult)
                nc.vector.tensor_sub(re, re, tv)
                nc.vector.tensor_tensor(tv, qe, stv, ALU.mult)
                nc.vector.tensor_tensor(ro, qo, ctv, ALU.mult)
                nc.vector.tensor_add(ro, ro, tv)
                # L2 norm over each head's (now contiguous) D slice
                sqs = nrm.tile([P, GD], BF16, tag="sqs")
                acc = nrm.tile([P, HPG], F32, tag="acc")
                for h in range(HPG):
                    nc.scalar.activation(sqs[:, h * D:(h + 1) * D], q2[:, h * D:(h + 1) * D],
                                         AF.Square, accum_out=acc[:, h:h + 1])
                nrm_t = nrm.tile([P, HPG], F32, tag="nrm_t")
                nc.scalar.activation(nrm_t[:], acc[:], AF.Sqrt)
                nc.vector.tensor_scalar_max(nrm_t[:], nrm_t[:], 1e-12)
                rcp = nrm.tile([P, HPG], F32, tag="rcp")
                nc.vector.reciprocal(rcp[:], nrm_t[:])
                if use_ls:
                    nc.vector.tensor_mul(rcp[:], rcp[:], ls_t[:])
                q3 = nrm.tile([P, GD], BF16, tag="q3")
                for h in range(HPG):
                    nc.vector.tensor_scalar_mul(q3[:, h * D:(h + 1) * D],
                                                q2[:, h * D:(h + 1) * D], rcp[:, h:h + 1])
                for h in range(HPG):
                    pt = pst.tile([P, P], BF16, tag="pt", name=f"pt_{lt}_{h}")
                    nc.tensor.matmul(pt[:], q3[:, h * D:(h + 1) * D], identb[:],
                                     is_transpose=True)
                    nc.any.tensor_copy(dstT[h][:, lt * P:(lt + 1) * P], pt[:])

            # V and K phases over all l-tiles
            for lt in range(NLT):
                psum = proj_tile(lt, d['xkvT'], w_all['wvT'])
                nc.scalar.copy(v_sb[:, lt, :], psum[:])
            for lt in range(NLT):
                psum = proj_tile(lt, d['xkvT'], w_all['wkT'])
                qk_norm(lt, psum, d['c4k'], d['s4k'], ks_t, False, kT)

            for lt in range(NLT):
                psum = proj_tile(lt, d['xqT'], w_all['wqT'])
                qk_norm(lt, psum, d['c4q'], d['s4q'], qs_t, True, qT)
            proj_ctx.close()

            # attention per q-chunk
            att_ctx = ExitStack()
            asb = att_ctx.enter_context(tc.tile_pool(name="att_sb", bufs=3))
            atp = att_ctx.enter_context(tc.tile_pool(name="att_at", bufs=1))
            aop = att_ctx.enter_context(tc.tile_pool(name="att_o", bufs=3))
            ps_pv = att_ctx.enter_context(tc.tile_pool(name="ps_pv", bufs=1, space="PSUM"))
            ps_s = att_ctx.enter_context(tc.tile_pool(name="ps_s", bufs=3, space="PSUM"))
            ps_d = att_ctx.enter_context(tc.tile_pool(name="ps_d", bufs=1, space="PSUM"))
            for qc in range(NQC):
                qsl = slice(qc * 512, (qc + 1) * 512)
                pv = [ps_pv.tile([P, 512], F32, tag=f"pv{h}", name=f"pv{qc}_{h}")
                      for h in range(HPG)]
                den = ps_d.tile([P, 512], F32, tag="den", name=f"den{qc}")
                def stage1(kt):
                    eb = asb.tile([P, 512], BF16, tag="eb", name=f"eb{qc}_{kt}")
                    nc.sync.dma_start(eb[:], d['expBT'][kt * P:(kt + 1) * P, qsl])
                    p_t = asb.tile([P, 2048], BF16, tag="p_t", name=f"pt{qc}_{kt}", bufs=3)
                    ebb = eb[:].rearrange("p (o q) -> p o q", o=1).broadcast_to([P, 2, 512])
                    for hp in range(2):
                        for i in range(2):
                            h = 2 * hp + i
                            s_ps = ps_s.tile([P, 512], F32, tag="s_ps", name=f"sp{qc}_{kt}_{h}")
                            nc.tensor.matmul(s_ps[:], kT[h][:, kt * P:(kt + 1) * P],
                                             qT[h][:, qsl], start=True, stop=True)
                            nc.scalar.activation(p_t[:, h * 512:(h + 1) * 512], s_ps[:], AF.Exp)
                        half = p_t[:, hp * 1024:(hp + 1) * 1024].rearrange("p (i q) -> p i q", i=2)
                        nc.vector.tensor_tensor(half, half, ebb, ALU.mult)
                    return p_t

                def stage2(kt, p_t):
                    for h in range(HPG):
                        nc.tensor.matmul(pv[h][:], v_sb[:, kt, h * D:(h + 1) * D],
                                         p_t[:, h * 512:(h + 1) * 512],
                                         start=(kt == 0), stop=(kt == NKT - 1))
                    for h in range(HPG):
                        nc.tensor.matmul(den[32 * h:32 * h + 32, :],
                                         ones_b[:, 0:32],
                                         p_t[:, h * 512:(h + 1) * 512],
                                         start=(kt == 0), stop=(kt == NKT - 1),
                                         tile_position=(0, 32 * h))

                prev = None
                for kt in range(NKT):
                    p_t = stage1(kt)
                    if prev is not None:
                        stage2(prev[0], prev[1])
                    prev = (kt, p_t)
                stage2(prev[0], prev[1])
                attn = [atp.tile([P, 512], BF16, tag=f"at{h}", name=f"at{qc}_{h}")
                        for h in range(HPG)]
                pvc = []
                for h in range(HPG):
                    c = asb.tile([P, 512], BF16, tag=f"pvc{h}", name=f"pvc{qc}_{h}", bufs=2)
                    nc.vector.tensor_copy(c[:], pv[h][:])
                    pvc.append(c)
                rcp32s = []
                for i in range(2):
                    lnd = asb.tile([64, 512], F32, tag=f"lnd{i}", name=f"lnd{qc}_{i}")
                    nc.scalar.activation(lnd[:], den[64 * i:64 * i + 64, :], AF.Ln)
                    rcp32 = asb.tile([64, 512], F32R, tag=f"rcp32{i}", name=f"rcp32{qc}_{i}")
                    nc.scalar.activation(rcp32[:], lnd[:], AF.Exp, scale=-1.0)
                    rcp32s.append(rcp32)
                for h in range(HPG):
                    b_ps = ps_s.tile([P, 512], F32, tag="s_ps", name=f"b{qc}_{h}")
                    r32 = rcp32s[h // 2]
                    off = 32 * (h % 2)
                    nc.tensor.matmul(b_ps[:], ones_r[off:off + 1, :],
                                     r32[off:off + 1, :], start=True, stop=True)
                    rcpb = asb.tile([P, 512], BF16, tag="rcpb", name=f"rb{qc}_{h}")
                    nc.vector.tensor_copy(rcpb[:], b_ps[:])
                    nc.vector.tensor_mul(attn[h][:], pvc[h][:], rcpb[:])
                for lsub in range(4):
                    for ec in range(4):
                        o_ps = ps_s.tile([P, 512], F32, tag="s_ps", name=f"o{qc}_{lsub}_{ec}")
                        for h in range(HPG):
                            nc.tensor.matmul(o_ps[:], attn[h][:, lsub * P:(lsub + 1) * P],
                                             wo_sb[:, h, ec * 512:(ec + 1) * 512],
                                             start=(h == 0), stop=(h == HPG - 1))
                        o_sb = aop.tile([P, 512], F32, tag="o_sb", name=f"ob{qc}_{lsub}_{ec}")
                        nc.vector.tensor_copy(o_sb[:], o_ps[:])
                        nc.sync.dma_start(
                            out[qc * 512 + lsub * P: qc * 512 + (lsub + 1) * P,
                                ec * 512:(ec + 1) * 512], o_sb[:])
            att_ctx.close()
    nc.compile()
    return nc


# head-dim permutation: within each head, evens first then odds
_PERM = np.empty(GD, np.int64)
for _i in range(GD):
    _h, _j = divmod(_i, D)
    _par, _dd = divmod(_j, D // 2)
    _PERM[_i] = _h * D + 2 * _dd + _par


def _prepare(inputs):
    f32 = np.float32
    inputs_q = np.asarray(inputs["inputs_q"], f32)
    inputs_kv = np.asarray(inputs["inputs_kv"], f32)
    bias = np.asarray(inputs["bias"], f32).reshape(L, L)
    q_sin = np.asarray(inputs["q_sinusoids"], f32)
    k_sin = np.asarray(inputs["k_sinusoids"], f32)
    Wq = np.asarray(inputs["Wq"], f32)
    Wk = np.asarray(inputs["Wk"], f32)
    Wv = np.asarray(inputs["Wv"], f32)
    Wo = np.asarray(inputs["Wo"], f32)
    qns = np.asarray(inputs["q_norm_scale"], f32)
    kns = np.asarray(inputs["k_norm_scale"], f32)
    ls = np.asarray(inputs["logit_scale"], f32)

    apply_qs = not np.all(qns == 1.0)
    apply_ks = not np.all(kns == 1.0)

    bm = bias.max(axis=1, keepdims=True)
    expBT = np.ascontiguousarray(np.exp((bias - bm).T).astype(NP_BF16))
    ls_e = np.exp(np.minimum(ls, LOGIT_SCALE_MAX)).astype(f32)

    per_b = []
    for b in range(B):
        per_b.append(dict(
            xqT=np.ascontiguousarray(inputs_q[b].T.astype(NP_BF16)),
            xkvT=np.ascontiguousarray(inputs_kv[b].T.astype(NP_BF16)),
            c4q=np.ascontiguousarray(np.tile(q_sin[b][:, 0::2], (1, HPG))),
            s4q=np.ascontiguousarray(np.tile(q_sin[b][:, 1::2], (1, HPG))),
            c4k=np.ascontiguousarray(np.tile(k_sin[b][:, 0::2], (1, HPG))),
            s4k=np.ascontiguousarray(np.tile(k_sin[b][:, 1::2], (1, HPG))),
        ))
    per_g = []
    for g in range(G):
        rows = slice(g * GD, (g + 1) * GD)
        per_g.append(dict(
            wqT=np.ascontiguousarray(Wq[rows, :][_PERM, :].T.astype(NP_BF16)),
            wkT=np.ascontiguousarray(Wk[rows, :][_PERM, :].T.astype(NP_BF16)),
            wvT=np.ascontiguousarray(Wv[rows, :].T.astype(NP_BF16)),
            woS=np.ascontiguousarray(Wo[:, rows].T.astype(NP_BF16)),
            ls=np.broadcast_to(ls_e[g * HPG:(g + 1) * HPG][None, :], (P, HPG)).copy(),
        ))

    qs_bc = (np.broadcast_to(np.tile(qns, HPG)[_PERM][None, :], (P, GD)).copy()
             if apply_qs else None)
    ks_bc = (np.broadcast_to(np.tile(kns, HPG)[_PERM][None, :], (P, GD)).copy()
             if apply_ks else None)

    in_maps = []
    for c in range(8):
        b, g = divmod(c, G)
        m = dict(expBT=expBT)
        m.update(per_b[b])
        m.update(per_g[g])
        if apply_qs:
            m['qscale'] = qs_bc
        if apply_ks:
            m['kscale'] = ks_bc
        in_maps.append(m)
    return in_maps, apply_qs, apply_ks


_CACHE = {}


def _get_nc(apply_qs, apply_ks):
    key = (apply_qs, apply_ks)
    if key not in _CACHE:
        _CACHE[key] = _build(apply_qs, apply_ks)
    return _CACHE[key]


def kernel(**inputs) -> np.ndarray:
    in_maps, apply_qs, apply_ks = _prepare(inputs)
    nc = _get_nc(apply_qs, apply_ks)
    res = run_bass_kernel_spmd(nc, in_maps, core_ids=list(range(8)))
    out = np.zeros((B, L, E), np.float32)
    for c in range(8):
        b = c // G
        out[b] += res.results[c]["out"]
    return out


# revision 21
# speedup vs baseline: 1.1130x; 1.0114x over previous
"""Multi-head scaled-cosine attention (B=2, L=2048, E=2048, H=16, D=128) on 8 trn2 cores.

Sharding: core c = (b, g) with b = batch (2), g = head-group of 4 heads (4 groups).
Each core computes its 4 heads' attention for its batch plus the partial output
projection; the host sums the 4 per-group partials per batch.

Precision: matmuls run in bf16 (PSUM accumulation is fp32); the softmax
denominator is accumulated in float32r and reduced across partitions with a
ones-matmul. The Q/K RMS-norm cancels exactly under the subsequent L2
normalization; the L2 reciprocal (and logit scale) are folded into a diagonal
matrix applied by the PE transpose that produces Q^T/K^T. exp(bias - rowmax) is
precomputed on the host and folded in multiplicatively. Scores are built
directly in [k, q] orientation so softmax and attn@V need no on-chip transpose
of the probability matrix. Q/K head dims are host-permuted (evens|odds) so RoPE
uses contiguous vector ops; the permutation cancels inside q.k.
"""
import sys
sys.path.insert(0, '/opt/trn_rl_repo')
import math
import numpy as np
import ml_dtypes

import concourse.bacc as bacc
import concourse.mybir as mybir
import concourse.tile as tile
from concourse.bass_utils import run_bass_kernel_spmd

F32 = mybir.dt.float32
F32R = mybir.dt.float32r
BF16 = mybir.dt.bfloat16
NP_BF16 = ml_dtypes.bfloat16
ALU = mybir.AluOpType
AF = mybir.ActivationFunctionType

B, L, E, H, D = 2, 2048, 2048, 16, 128
G = 4                 # head groups
HPG = H // G          # heads per group = 4
GD = HPG * D          # 512, per-group projection width
P = 128               # partitions
NLT = L // P          # 16 l-tiles
NET = E // P          # 16 e-tiles (contraction)
NQC = L // 512        # 4 q-chunks
NKT = L // P          # 16 k-tiles
HD2 = GD // 2         # 256
LOGIT_SCALE_MAX = math.log(1.0 / 0.01)


def _build(apply_qs: bool, apply_ks: bool):
    nc = bacc.Bacc(None, target_bir_lowering=False)
    d = {}
    d['xqT'] = nc.dram_tensor("xqT", [E, L], BF16, kind="ExternalInput")
    d['xkvT'] = nc.dram_tensor("xkvT", [E, L], BF16, kind="ExternalInput")
    d['expBT'] = nc.dram_tensor("expBT", [L, L], BF16, kind="ExternalInput")
    d['wqT'] = nc.dram_tensor("wqT", [E, GD], BF16, kind="ExternalInput")
    d['wkT'] = nc.dram_tensor("wkT", [E, GD], BF16, kind="ExternalInput")
    d['wvT'] = nc.dram_tensor("wvT", [E, GD], BF16, kind="ExternalInput")
    d['woS'] = nc.dram_tensor("woS", [GD, E], BF16, kind="ExternalInput")
    d['c4q'] = nc.dram_tensor("c4q", [L, HD2], F32, kind="ExternalInput")
    d['s4q'] = nc.dram_tensor("s4q", [L, HD2], F32, kind="ExternalInput")
    d['c4k'] = nc.dram_tensor("c4k", [L, HD2], F32, kind="ExternalInput")
    d['s4k'] = nc.dram_tensor("s4k", [L, HD2], F32, kind="ExternalInput")
    d['ls'] = nc.dram_tensor("ls", [P, HPG], F32, kind="ExternalInput")
    if apply_qs:
        d['qscale'] = nc.dram_tensor("qscale", [P, GD], F32, kind="ExternalInput")
    if apply_ks:
        d['kscale'] = nc.dram_tensor("kscale", [P, GD], F32, kind="ExternalInput")
    out = nc.dram_tensor("out", [L, E], F32, kind="ExternalOutput")

    with tile.TileContext(nc) as tc:
        with tc.tile_pool(name="persist", bufs=1) as persist:
            qT = [persist.tile([P, L], BF16, tag=f"qT{h}", name=f"qT{h}") for h in range(HPG)]
            kT = [persist.tile([P, L], BF16, tag=f"kT{h}", name=f"kT{h}") for h in range(HPG)]
            v_sb = persist.tile([P, NLT, GD], BF16, tag="v_sb")
            identb = persist.tile([P, P], BF16, tag="identb")
            identf = persist.tile([P, P], F32, tag="identf")
            nc.vector.memset(identf[:], 0.0)
            nc.gpsimd.affine_select(out=identf[:], in_=identf[:],
                                    compare_op=ALU.not_equal, fill=1.0, base=0,
                                    pattern=[[-1, P]], channel_multiplier=1)
            nc.vector.tensor_copy(identb[:], identf[:])
            ones_f = persist.tile([P, P], F32, tag="ones_f")
            nc.vector.memset(ones_f[:], 1.0)
            ones_r = persist.tile([P, P], F32R, tag="ones_r")
            nc.scalar.copy(ones_r[:], ones_f[:])
            ones_b = persist.tile([P, P], BF16, tag="ones_b")
            nc.vector.tensor_copy(ones_b[:], ones_f[:])
            ls_t = persist.tile([P, HPG], F32, tag="ls_t")
            nc.sync.dma_start(ls_t[:], d['ls'][:])
            w_all = {}
            for wname in ('wvT', 'wkT', 'wqT'):
                w_all[wname] = persist.tile([P, NET, GD], BF16, tag=wname, name=f"w_{wname}")
                nc.sync.dma_start(
                    w_all[wname][:], d[wname][:].rearrange("(e p) n -> p e n", p=P))
            wo_sb = persist.tile([P, HPG, E], BF16, tag="wo_sb")
            nc.sync.dma_start(
                wo_sb[:], d['woS'][:].rearrange("(h p) e -> p h e", p=P))

            qs_t = ks_t = None
            if apply_qs:
                qs_t = persist.tile([P, GD], F32, tag="qs_t")
                nc.sync.dma_start(qs_t[:], d['qscale'][:])
            if apply_ks:
                ks_t = persist.tile([P, GD], F32, tag="ks_t")
                nc.sync.dma_start(ks_t[:], d['kscale'][:])

            from contextlib import ExitStack
            proj_ctx = ExitStack()
            sbp = proj_ctx.enter_context(tc.tile_pool(name="proj_sb", bufs=3))
            nrm = proj_ctx.enter_context(tc.tile_pool(name="proj_nrm", bufs=4))
            psp = proj_ctx.enter_context(tc.tile_pool(name="proj_ps", bufs=2, space="PSUM"))
            pst = proj_ctx.enter_context(tc.tile_pool(name="proj_pst", bufs=2, space="PSUM"))

            def proj_tile(lt, x_dram, w_sb):
                blk = sbp.tile([P, NET, P], BF16, tag="xblk", name=f"xblk_{lt}")
                nc.sync.dma_start(
                    blk[:],
                    x_dram[:, lt * P:(lt + 1) * P].rearrange("(g p) l -> p g l", p=P))
                psum = psp.tile([P, GD], F32, tag="psum", name=f"psum_{lt}")
                for e in range(NET):
                    nc.tensor.matmul(psum[:], blk[:, e, :], w_sb[:, e, :],
                                     start=(e == 0), stop=(e == NET - 1))
                return psum

            def qk_norm(lt, psum, c_dram, s_dram, scale_tile, use_ls, dstT):
                q1 = nrm.tile([P, GD], BF16, tag="q1")
                nc.scalar.copy(q1[:], psum[:])
                if scale_tile is not None:
                    nc.vector.tensor_mul(q1[:], q1[:], scale_tile[:])
                ct = nrm.tile([P, HD2], F32, tag="ct")
                st = nrm.tile([P, HD2], F32, tag="st")
                nc.sync.dma_start(ct[:], c_dram[lt * P:(lt + 1) * P, :])
                nc.sync.dma_start(st[:], s_dram[lt * P:(lt + 1) * P, :])
                # per-head layout [evens(64) | odds(64)] (host-permuted weights)
                q1v = q1[:].rearrange("p (hh par dd) -> p hh par dd", hh=HPG, par=2)
                qe, qo = q1v[:, :, 0, :], q1v[:, :, 1, :]
                q2 = nrm.tile([P, GD], BF16, tag="q2")
                q2v = q2[:].rearrange("p (hh par dd) -> p hh par dd", hh=HPG, par=2)
                re, ro = q2v[:, :, 0, :], q2v[:, :, 1, :]
                ctv = ct[:].rearrange("p (hh dd) -> p hh dd", hh=HPG)
                stv = st[:].rearrange("p (hh dd) -> p hh dd", hh=HPG)
                tmp = nrm.tile([P, HD2], BF16, tag="tmp")
                tv = tmp[:].rearrange("p (hh dd) -> p hh dd", hh=HPG)
                # evens: qe*c - qo*s ; odds: qo*c + qe*s
                nc.vector.tensor_tensor(tv, qo, stv, ALU.mult)
                nc.vector.tensor_tensor(re, qe, ctv, ALU.mult)
                nc.vector.tensor_sub(re, re, tv)
                nc.vector.tensor_tensor(tv, qe, stv, ALU.mult)
                nc.vector.tensor_tensor(ro, qo, ctv, ALU.mult)
                nc.vector.tensor_add(ro, ro, tv)
                # L2 norm over each head's (now contiguous) D slice
                sqs = nrm.tile([P, GD], BF16, tag="sqs")
                acc = nrm.tile([P, HPG], F32, tag="acc")
                for h in range(HPG):
                    nc.scalar.activation(sqs[:, h * D:(h + 1) * D], q2[:, h * D:(h + 1) * D],
                                         AF.Square, accum_out=acc[:, h:h + 1])
                nrm_t = nrm.tile([P, HPG], F32, tag="nrm_t")
                nc.scalar.activation(nrm_t[:], acc[:], AF.Sqrt)
                nc.vector.tensor_scalar_max(nrm_t[:], nrm_t[:], 1e-12)
                rcp = nrm.tile([P, HPG], F32, tag="rcp")
                nc.vector.reciprocal(rcp[:], nrm_t[:])
                if use_ls:
                    nc.vector.tensor_mul(rcp[:], rcp[:], ls_t[:])
                q3 = nrm.tile([P, GD], BF16, tag="q3")
                for h in range(HPG):
                    nc.vector.tensor_scalar_mul(q3[:, h * D:(h + 1) * D],
                                                q2[:, h * D:(h + 1) * D], rcp[:, h:h + 1])
                for h in range(HPG):
                    pt = pst.tile([P, P], BF16, tag="pt", name=f"pt_{lt}_{h}")
                    nc.tensor.matmul(pt[:], q3[:, h * D:(h + 1) * D], identb[:],
                                     is_transpose=True)
                    nc.any.tensor_copy(dstT[h][:, lt * P:(lt + 1) * P], pt[:])

            # merged V+K phase: one xkvT block load feeds both projections
            for lt in range(NLT):
                blk = sbp.tile([P, NET, P], BF16, tag="xblk", name=f"xkvblk_{lt}")
                nc.sync.dma_start(
                    blk[:],
                    d['xkvT'][:, lt * P:(lt + 1) * P].rearrange("(g p) l -> p g l", p=P))
                psum_v = psp.tile([P, GD], F32, tag="psum", name=f"psumv_{lt}")
                for e in range(NET):
                    nc.tensor.matmul(psum_v[:], blk[:, e, :], w_all['wvT'][:, e, :],
                                     start=(e == 0), stop=(e == NET - 1))
                nc.scalar.copy(v_sb[:, lt, :], psum_v[:])
                psum_k = psp.tile([P, GD], F32, tag="psum", name=f"psumk_{lt}")
                for e in range(NET):
                    nc.tensor.matmul(psum_k[:], blk[:, e, :], w_all['wkT'][:, e, :],
                                     start=(e == 0), stop=(e == NET - 1))
                qk_norm(lt, psum_k, d['c4k'], d['s4k'], ks_t, False, kT)

            for lt in range(NLT):
                psum = proj_tile(lt, d['xqT'], w_all['wqT'])
                qk_norm(lt, psum, d['c4q'], d['s4q'], qs_t, True, qT)
            proj_ctx.close()

            # attention per q-chunk
            att_ctx = ExitStack()
            asb = att_ctx.enter_context(tc.tile_pool(name="att_sb", bufs=3))
            atp = att_ctx.enter_context(tc.tile_pool(name="att_at", bufs=1))
            aop = att_ctx.enter_context(tc.tile_pool(name="att_o", bufs=3))
            ps_pv = att_ctx.enter_context(tc.tile_pool(name="ps_pv", bufs=1, space="PSUM"))
            ps_s = att_ctx.enter_context(tc.tile_pool(name="ps_s", bufs=3, space="PSUM"))
            ps_d = att_ctx.enter_context(tc.tile_pool(name="ps_d", bufs=1, space="PSUM"))
            pending = []
            for qc in range(NQC):
                qsl = slice(qc * 512, (qc + 1) * 512)
                pv = [ps_pv.tile([P, 512], F32, tag=f"pv{h}", name=f"pv{qc}_{h}")
                      for h in range(HPG)]
                den = ps_d.tile([P, 512], F32, tag="den", name=f"den{qc}")
                def stage1(kt):
                    eb = asb.tile([P, 512], BF16, tag="eb", name=f"eb{qc}_{kt}")
                    nc.sync.dma_start(eb[:], d['expBT'][kt * P:(kt + 1) * P, qsl])
                    p_t = asb.tile([P, 2048], BF16, tag="p_t", name=f"pt{qc}_{kt}", bufs=3)
                    ebb = eb[:].rearrange("p (o q) -> p o q", o=1).broadcast_to([P, 2, 512])
                    for hp in range(2):
                        for i in range(2):
                            h = 2 * hp + i
                            s_ps = ps_s.tile([P, 512], F32, tag="s_ps", name=f"sp{qc}_{kt}_{h}")
                            nc.tensor.matmul(s_ps[:], kT[h][:, kt * P:(kt + 1) * P],
                                             qT[h][:, qsl], start=True, stop=True)
                            nc.scalar.activation(p_t[:, h * 512:(h + 1) * 512], s_ps[:], AF.Exp)
                        half = p_t[:, hp * 1024:(hp + 1) * 1024].rearrange("p (i q) -> p i q", i=2)
                        nc.vector.tensor_tensor(half, half, ebb, ALU.mult)
                    return p_t

                def stage2(kt, p_t):
                    for h in range(HPG):
                        nc.tensor.matmul(pv[h][:], v_sb[:, kt, h * D:(h + 1) * D],
                                         p_t[:, h * 512:(h + 1) * 512],
                                         start=(kt == 0), stop=(kt == NKT - 1))
                    for h in range(HPG):
                        nc.tensor.matmul(den[32 * h:32 * h + 32, :],
                                         ones_b[:, 0:32],
                                         p_t[:, h * 512:(h + 1) * 512],
                                         start=(kt == 0), stop=(kt == NKT - 1),
                                         tile_position=(0, 32 * h))

                prev = None
                for kt in range(NKT):
                    p_t = stage1(kt)
                    if prev is not None:
                        stage2(prev[0], prev[1])
                    prev = (kt, p_t)
                stage2(prev[0], prev[1])
                attn = [atp.tile([P, 512], BF16, tag=f"at{h}", name=f"at{qc}_{h}", bufs=2)
                        for h in range(HPG)]
                pvc = []
                for h in range(HPG):
                    c = asb.tile([P, 512], BF16, tag=f"pvc{h}", name=f"pvc{qc}_{h}", bufs=2)
                    nc.vector.tensor_copy(c[:], pv[h][:])
                    pvc.append(c)
                rcp32s = []
                for i in range(2):
                    lnd = asb.tile([64, 512], F32, tag=f"lnd{i}", name=f"lnd{qc}_{i}")
                    nc.scalar.activation(lnd[:], den[64 * i:64 * i + 64, :], AF.Ln)
                    rcp32 = asb.tile([64, 512], F32R, tag=f"rcp32{i}", name=f"rcp32{qc}_{i}")
                    nc.scalar.activation(rcp32[:], lnd[:], AF.Exp, scale=-1.0)
                    rcp32s.append(rcp32)
                for h in range(HPG):
                    b_ps = ps_s.tile([P, 512], F32, tag="s_ps", name=f"b{qc}_{h}")
                    r32 = rcp32s[h // 2]
                    off = 32 * (h % 2)
                    nc.tensor.matmul(b_ps[:], ones_r[off:off + 1, :],
                                     r32[off:off + 1, :], start=True, stop=True)
                    rcpb = asb.tile([P, 512], BF16, tag="rcpb", name=f"rb{qc}_{h}")
                    nc.vector.tensor_copy(rcpb[:], b_ps[:])
                    nc.vector.tensor_mul(attn[h][:], pvc[h][:], rcpb[:])
                def outproj(qc, attn):
                    for lsub in range(4):
                        for ec in range(4):
                            o_ps = ps_s.tile([P, 512], F32, tag="s_ps", name=f"o{qc}_{lsub}_{ec}")
                            for h in range(HPG):
                                nc.tensor.matmul(o_ps[:], attn[h][:, lsub * P:(lsub + 1) * P],
                                                 wo_sb[:, h, ec * 512:(ec + 1) * 512],
                                                 start=(h == 0), stop=(h == HPG - 1))
                            o_sb = aop.tile([P, 512], F32, tag="o_sb", name=f"ob{qc}_{lsub}_{ec}")
                            nc.vector.tensor_copy(o_sb[:], o_ps[:])
                            nc.sync.dma_start(
                                out[qc * 512 + lsub * P: qc * 512 + (lsub + 1) * P,
                                    ec * 512:(ec + 1) * 512], o_sb[:])
                pending.append((qc, attn))
                if len(pending) > 1:
                    outproj(*pending.pop(0))
            for item in pending:
                outproj(*item)
            att_ctx.close()
    nc.compile()
    return nc


# head-dim permutation: within each head, evens first then odds
_PERM = np.empty(GD, np.int64)
for _i in range(GD):
    _h, _j = divmod(_i, D)
    _par, _dd = divmod(_j, D // 2)
    _PERM[_i] = _h * D + 2 * _dd + _par


def _prepare(inputs):
    f32 = np.float32
    inputs_q = np.asarray(inputs["inputs_q"], f32)
    inputs_kv = np.asarray(inputs["inputs_kv"], f32)
    bias = np.asarray(inputs["bias"], f32).reshape(L, L)
    q_sin = np.asarray(inputs["q_sinusoids"], f32)
    k_sin = np.asarray(inputs["k_sinusoids"], f32)
    Wq = np.asarray(inputs["Wq"], f32)
    Wk = np.asarray(inputs["Wk"], f32)
    Wv = np.asarray(inputs["Wv"], f32)
    Wo = np.asarray(inputs["Wo"], f32)
    qns = np.asarray(inputs["q_norm_scale"], f32)
    kns = np.asarray(inputs["k_norm_scale"], f32)
    ls = np.asarray(inputs["logit_scale"], f32)

    apply_qs = not np.all(qns == 1.0)
    apply_ks = not np.all(kns == 1.0)

    bm = bias.max(axis=1, keepdims=True)
    expBT = np.ascontiguousarray(np.exp((bias - bm).T).astype(NP_BF16))
    ls_e = np.exp(np.minimum(ls, LOGIT_SCALE_MAX)).astype(f32)

    per_b = []
    for b in range(B):
        per_b.append(dict(
            xqT=np.ascontiguousarray(inputs_q[b].T.astype(NP_BF16)),
            xkvT=np.ascontiguousarray(inputs_kv[b].T.astype(NP_BF16)),
            c4q=np.ascontiguousarray(np.tile(q_sin[b][:, 0::2], (1, HPG))),
            s4q=np.ascontiguousarray(np.tile(q_sin[b][:, 1::2], (1, HPG))),
            c4k=np.ascontiguousarray(np.tile(k_sin[b][:, 0::2], (1, HPG))),
            s4k=np.ascontiguousarray(np.tile(k_sin[b][:, 1::2], (1, HPG))),
        ))
    per_g = []
    for g in range(G):
        rows = slice(g * GD, (g + 1) * GD)
        per_g.append(dict(
            wqT=np.ascontiguousarray(Wq[rows, :][_PERM, :].T.astype(NP_BF16)),
            wkT=np.ascontiguousarray(Wk[rows, :][_PERM, :].T.astype(NP_BF16)),
            wvT=np.ascontiguousarray(Wv[rows, :].T.astype(NP_BF16)),
            woS=np.ascontiguousarray(Wo[:, rows].T.astype(NP_BF16)),
            ls=np.broadcast_to(ls_e[g * HPG:(g + 1) * HPG][None, :], (P, HPG)).copy(),
        ))

    qs_bc = (np.broadcast_to(np.tile(qns, HPG)[_PERM][None, :], (P, GD)).copy()
             if apply_qs else None)
    ks_bc = (np.broadcast_to(np.tile(kns, HPG)[_PERM][None, :], (P, GD)).copy()
             if apply_ks else None)

    in_maps = []
    for c in range(8):
        b, g = divmod(c, G)
        m = dict(expBT=expBT)
        m.update(per_b[b])
        m.update(per_g[g])
        if apply_qs:
            m['qscale'] = qs_bc
        if apply_ks:
            m['kscale'] = ks_bc
        in_maps.append(m)
    return in_maps, apply_qs, apply_ks


_CACHE = {}


def _get_nc(apply_qs, apply_ks):
    key = (apply_qs, apply_ks)
    if key not in _CACHE:
        _CACHE[key] = _build(apply_qs, apply_ks)
    return _CACHE[key]


def kernel(**inputs) -> np.ndarray:
    in_maps, apply_qs, apply_ks = _prepare(inputs)
    nc = _get_nc(apply_qs, apply_ks)
    res = run_bass_kernel_spmd(nc, in_maps, core_ids=list(range(8)))
    out = np.zeros((B, L, E), np.float32)
    for c in range(8):
        b = c // G
        out[b] += res.results[c]["out"]
    return out


# revision 22
# speedup vs baseline: 1.1255x; 1.0113x over previous
"""Multi-head scaled-cosine attention (B=2, L=2048, E=2048, H=16, D=128) on 8 trn2 cores.

Sharding: core c = (b, g) with b = batch (2), g = head-group of 4 heads (4 groups).
Each core computes its 4 heads' attention for its batch plus the partial output
projection; the host sums the 4 per-group partials per batch.

Precision: matmuls run in bf16 (PSUM accumulation is fp32); the softmax
denominator is accumulated in float32r and reduced across partitions with a
ones-matmul. The Q/K RMS-norm cancels exactly under the subsequent L2
normalization; the L2 reciprocal (and logit scale) are folded into a diagonal
matrix applied by the PE transpose that produces Q^T/K^T. exp(bias - rowmax) is
precomputed on the host and folded in multiplicatively. Scores are built
directly in [k, q] orientation so softmax and attn@V need no on-chip transpose
of the probability matrix. Q/K head dims are host-permuted (evens|odds) so RoPE
uses contiguous vector ops; the permutation cancels inside q.k.
"""
import sys
sys.path.insert(0, '/opt/trn_rl_repo')
import math
import numpy as np
import ml_dtypes

import concourse.bacc as bacc
import concourse.mybir as mybir
import concourse.tile as tile
from concourse.bass_utils import run_bass_kernel_spmd

F32 = mybir.dt.float32
F32R = mybir.dt.float32r
BF16 = mybir.dt.bfloat16
NP_BF16 = ml_dtypes.bfloat16
ALU = mybir.AluOpType
AF = mybir.ActivationFunctionType

B, L, E, H, D = 2, 2048, 2048, 16, 128
G = 4                 # head groups
HPG = H // G          # heads per group = 4
GD = HPG * D          # 512, per-group projection width
P = 128               # partitions
NLT = L // P          # 16 l-tiles
NET = E // P          # 16 e-tiles (contraction)
NQC = L // 512        # 4 q-chunks
NKT = L // P          # 16 k-tiles
HD2 = GD // 2         # 256
LOGIT_SCALE_MAX = math.log(1.0 / 0.01)


def _build(apply_qs: bool, apply_ks: bool):
    nc = bacc.Bacc(None, target_bir_lowering=False)
    d = {}
    d['xqT'] = nc.dram_tensor("xqT", [E, L], BF16, kind="ExternalInput")
    d['xkvT'] = nc.dram_tensor("xkvT", [E, L], BF16, kind="ExternalInput")
    d['expBT'] = nc.dram_tensor("expBT", [L, L], BF16, kind="ExternalInput")
    d['wqT'] = nc.dram_tensor("wqT", [E, GD], BF16, kind="ExternalInput")
    d['wkT'] = nc.dram_tensor("wkT", [E, GD], BF16, kind="ExternalInput")
    d['wvT'] = nc.dram_tensor("wvT", [E, GD], BF16, kind="ExternalInput")
    d['woS'] = nc.dram_tensor("woS", [GD, E], BF16, kind="ExternalInput")
    d['c4q'] = nc.dram_tensor("c4q", [L, HD2], F32, kind="ExternalInput")
    d['s4q'] = nc.dram_tensor("s4q", [L, HD2], F32, kind="ExternalInput")
    d['c4k'] = nc.dram_tensor("c4k", [L, HD2], F32, kind="ExternalInput")
    d['s4k'] = nc.dram_tensor("s4k", [L, HD2], F32, kind="ExternalInput")
    d['ls'] = nc.dram_tensor("ls", [P, HPG], F32, kind="ExternalInput")
    if apply_qs:
        d['qscale'] = nc.dram_tensor("qscale", [P, GD], F32, kind="ExternalInput")
    if apply_ks:
        d['kscale'] = nc.dram_tensor("kscale", [P, GD], F32, kind="ExternalInput")
    out = nc.dram_tensor("out", [L, E], F32, kind="ExternalOutput")

    with tile.TileContext(nc) as tc:
        with tc.tile_pool(name="persist", bufs=1) as persist:
            qT = [persist.tile([P, L], BF16, tag=f"qT{h}", name=f"qT{h}") for h in range(HPG)]
            kT = [persist.tile([P, L], BF16, tag=f"kT{h}", name=f"kT{h}") for h in range(HPG)]
            v_sb = persist.tile([P, NLT, GD], BF16, tag="v_sb")
            identb = persist.tile([P, P], BF16, tag="identb")
            identf = persist.tile([P, P], F32, tag="identf")
            nc.vector.memset(identf[:], 0.0)
            nc.gpsimd.affine_select(out=identf[:], in_=identf[:],
                                    compare_op=ALU.not_equal, fill=1.0, base=0,
                                    pattern=[[-1, P]], channel_multiplier=1)
            nc.vector.tensor_copy(identb[:], identf[:])
            ones_f = persist.tile([P, P], F32, tag="ones_f")
            nc.vector.memset(ones_f[:], 1.0)
            ones_r = persist.tile([P, P], F32R, tag="ones_r")
            nc.scalar.copy(ones_r[:], ones_f[:])
            ones_b = persist.tile([P, P], BF16, tag="ones_b")
            nc.vector.tensor_copy(ones_b[:], ones_f[:])
            ls_t = persist.tile([P, HPG], F32, tag="ls_t")
            nc.sync.dma_start(ls_t[:], d['ls'][:])
            w_all = {}
            for wname in ('wvT', 'wkT', 'wqT'):
                w_all[wname] = persist.tile([P, NET, GD], BF16, tag=wname, name=f"w_{wname}")
                nc.sync.dma_start(
                    w_all[wname][:], d[wname][:].rearrange("(e p) n -> p e n", p=P))
            wo_sb = persist.tile([P, HPG, E], BF16, tag="wo_sb")
            nc.sync.dma_start(
                wo_sb[:], d['woS'][:].rearrange("(h p) e -> p h e", p=P))

            qs_t = ks_t = None
            if apply_qs:
                qs_t = persist.tile([P, GD], F32, tag="qs_t")
                nc.sync.dma_start(qs_t[:], d['qscale'][:])
            if apply_ks:
                ks_t = persist.tile([P, GD], F32, tag="ks_t")
                nc.sync.dma_start(ks_t[:], d['kscale'][:])

            from contextlib import ExitStack
            proj_ctx = ExitStack()
            sbp = proj_ctx.enter_context(tc.tile_pool(name="proj_sb", bufs=4))
            nrm = proj_ctx.enter_context(tc.tile_pool(name="proj_nrm", bufs=6))
            psp = proj_ctx.enter_context(tc.tile_pool(name="proj_ps", bufs=3, space="PSUM"))
            pst = proj_ctx.enter_context(tc.tile_pool(name="proj_pst", bufs=3, space="PSUM"))

            def proj_tile(lt, x_dram, w_sb):
                blk = sbp.tile([P, NET, P], BF16, tag="xblk", name=f"xblk_{lt}")
                nc.sync.dma_start(
                    blk[:],
                    x_dram[:, lt * P:(lt + 1) * P].rearrange("(g p) l -> p g l", p=P))
                psum = psp.tile([P, GD], F32, tag="psum", name=f"psum_{lt}")
                for e in range(NET):
                    nc.tensor.matmul(psum[:], blk[:, e, :], w_sb[:, e, :],
                                     start=(e == 0), stop=(e == NET - 1))
                return psum

            def qk_norm(lt, psum, c_dram, s_dram, scale_tile, use_ls, dstT):
                q1 = nrm.tile([P, GD], BF16, tag="q1")
                nc.scalar.copy(q1[:], psum[:])
                if scale_tile is not None:
                    nc.vector.tensor_mul(q1[:], q1[:], scale_tile[:])
                ct = nrm.tile([P, HD2], F32, tag="ct")
                st = nrm.tile([P, HD2], F32, tag="st")
                nc.sync.dma_start(ct[:], c_dram[lt * P:(lt + 1) * P, :])
                nc.sync.dma_start(st[:], s_dram[lt * P:(lt + 1) * P, :])
                # per-head layout [evens(64) | odds(64)] (host-permuted weights)
                q1v = q1[:].rearrange("p (hh par dd) -> p hh par dd", hh=HPG, par=2)
                qe, qo = q1v[:, :, 0, :], q1v[:, :, 1, :]
                q2 = nrm.tile([P, GD], BF16, tag="q2")
                q2v = q2[:].rearrange("p (hh par dd) -> p hh par dd", hh=HPG, par=2)
                re, ro = q2v[:, :, 0, :], q2v[:, :, 1, :]
                ctv = ct[:].rearrange("p (hh dd) -> p hh dd", hh=HPG)
                stv = st[:].rearrange("p (hh dd) -> p hh dd", hh=HPG)
                tmp = nrm.tile([P, HD2], BF16, tag="tmp")
                tv = tmp[:].rearrange("p (hh dd) -> p hh dd", hh=HPG)
                # evens: qe*c - qo*s ; odds: qo*c + qe*s
                nc.vector.tensor_tensor(tv, qo, stv, ALU.mult)
                nc.vector.tensor_tensor(re, qe, ctv, ALU.mult)
                nc.vector.tensor_sub(re, re, tv)
                nc.vector.tensor_tensor(tv, qe, stv, ALU.mult)
                nc.vector.tensor_tensor(ro, qo, ctv, ALU.mult)
                nc.vector.tensor_add(ro, ro, tv)
                # L2 norm over each head's (now contiguous) D slice
                sqs = nrm.tile([P, GD], BF16, tag="sqs")
                acc = nrm.tile([P, HPG], F32, tag="acc")
                for h in range(HPG):
                    nc.scalar.activation(sqs[:, h * D:(h + 1) * D], q2[:, h * D:(h + 1) * D],
                                         AF.Square, accum_out=acc[:, h:h + 1])
                nrm_t = nrm.tile([P, HPG], F32, tag="nrm_t")
                nc.scalar.activation(nrm_t[:], acc[:], AF.Sqrt)
                nc.vector.tensor_scalar_max(nrm_t[:], nrm_t[:], 1e-12)
                rcp = nrm.tile([P, HPG], F32, tag="rcp")
                nc.vector.reciprocal(rcp[:], nrm_t[:])
                if use_ls:
                    nc.vector.tensor_mul(rcp[:], rcp[:], ls_t[:])
                q3 = nrm.tile([P, GD], BF16, tag="q3")
                for h in range(HPG):
                    nc.vector.tensor_scalar_mul(q3[:, h * D:(h + 1) * D],
                                                q2[:, h * D:(h + 1) * D], rcp[:, h:h + 1])
                for h in range(HPG):
                    pt = pst.tile([P, P], BF16, tag="pt", name=f"pt_{lt}_{h}")
                    nc.tensor.matmul(pt[:], q3[:, h * D:(h + 1) * D], identb[:],
                                     is_transpose=True)
                    nc.any.tensor_copy(dstT[h][:, lt * P:(lt + 1) * P], pt[:])

            # merged V+K phase: one xkvT block load feeds both projections
            for lt in range(NLT):
                blk = sbp.tile([P, NET, P], BF16, tag="xblk", name=f"xkvblk_{lt}")
                nc.sync.dma_start(
                    blk[:],
                    d['xkvT'][:, lt * P:(lt + 1) * P].rearrange("(g p) l -> p g l", p=P))
                psum_v = psp.tile([P, GD], F32, tag="psum", name=f"psumv_{lt}")
                for e in range(NET):
                    nc.tensor.matmul(psum_v[:], blk[:, e, :], w_all['wvT'][:, e, :],
                                     start=(e == 0), stop=(e == NET - 1))
                nc.scalar.copy(v_sb[:, lt, :], psum_v[:])
                psum_k = psp.tile([P, GD], F32, tag="psum", name=f"psumk_{lt}")
                for e in range(NET):
                    nc.tensor.matmul(psum_k[:], blk[:, e, :], w_all['wkT'][:, e, :],
                                     start=(e == 0), stop=(e == NET - 1))
                qk_norm(lt, psum_k, d['c4k'], d['s4k'], ks_t, False, kT)

            for lt in range(NLT):
                psum = proj_tile(lt, d['xqT'], w_all['wqT'])
                qk_norm(lt, psum, d['c4q'], d['s4q'], qs_t, True, qT)
            proj_ctx.close()

            # attention per q-chunk
            att_ctx = ExitStack()
            asb = att_ctx.enter_context(tc.tile_pool(name="att_sb", bufs=3))
            atp = att_ctx.enter_context(tc.tile_pool(name="att_at", bufs=1))
            aop = att_ctx.enter_context(tc.tile_pool(name="att_o", bufs=3))
            ps_pv = att_ctx.enter_context(tc.tile_pool(name="ps_pv", bufs=1, space="PSUM"))
            ps_s = att_ctx.enter_context(tc.tile_pool(name="ps_s", bufs=3, space="PSUM"))
            ps_d = att_ctx.enter_context(tc.tile_pool(name="ps_d", bufs=1, space="PSUM"))
            pending = []
            for qc in range(NQC):
                qsl = slice(qc * 512, (qc + 1) * 512)
                pv = [ps_pv.tile([P, 512], F32, tag=f"pv{h}", name=f"pv{qc}_{h}")
                      for h in range(HPG)]
                den = ps_d.tile([P, 512], F32, tag="den", name=f"den{qc}")
                def stage1(kt):
                    eb = asb.tile([P, 512], BF16, tag="eb", name=f"eb{qc}_{kt}")
                    nc.sync.dma_start(eb[:], d['expBT'][kt * P:(kt + 1) * P, qsl])
                    p_t = asb.tile([P, 2048], BF16, tag="p_t", name=f"pt{qc}_{kt}", bufs=3)
                    ebb = eb[:].rearrange("p (o q) -> p o q", o=1).broadcast_to([P, 2, 512])
                    for hp in range(2):
                        for i in range(2):
                            h = 2 * hp + i
                            s_ps = ps_s.tile([P, 512], F32, tag="s_ps", name=f"sp{qc}_{kt}_{h}")
                            nc.tensor.matmul(s_ps[:], kT[h][:, kt * P:(kt + 1) * P],
                                             qT[h][:, qsl], start=True, stop=True)
                            nc.scalar.activation(p_t[:, h * 512:(h + 1) * 512], s_ps[:], AF.Exp)
                        half = p_t[:, hp * 1024:(hp + 1) * 1024].rearrange("p (i q) -> p i q", i=2)
                        nc.vector.tensor_tensor(half, half, ebb, ALU.mult)
                    return p_t

                def stage2(kt, p_t):
                    for h in range(HPG):
                        nc.tensor.matmul(pv[h][:], v_sb[:, kt, h * D:(h + 1) * D],
                                         p_t[:, h * 512:(h + 1) * 512],
                                         start=(kt == 0), stop=(kt == NKT - 1))
                    for h in range(HPG):
                        nc.tensor.matmul(den[32 * h:32 * h + 32, :],
                                         ones_b[:, 0:32],
                                         p_t[:, h * 512:(h + 1) * 512],
                                         start=(kt == 0), stop=(kt == NKT - 1),
                                         tile_position=(0, 32 * h))

                prev = None
                for kt in range(NKT):
                    p_t = stage1(kt)
                    if prev is not None:
                        stage2(prev[0], prev[1])
                    prev = (kt, p_t)
                stage2(prev[0], prev[1])
                attn = [atp.tile([P, 512], BF16, tag=f"at{h}", name=f"at{qc}_{h}", bufs=2)
                        for h in range(HPG)]
                pvc = []
                for h in range(HPG):
                    c = asb.tile([P, 512], BF16, tag=f"pvc{h}", name=f"pvc{qc}_{h}", bufs=2)
                    nc.vector.tensor_copy(c[:], pv[h][:])
                    pvc.append(c)
                rcp32s = []
                for i in range(2):
                    lnd = asb.tile([64, 512], F32, tag=f"lnd{i}", name=f"lnd{qc}_{i}")
                    nc.scalar.activation(lnd[:], den[64 * i:64 * i + 64, :], AF.Ln)
                    rcp32 = asb.tile([64, 512], F32R, tag=f"rcp32{i}", name=f"rcp32{qc}_{i}")
                    nc.scalar.activation(rcp32[:], lnd[:], AF.Exp, scale=-1.0)
                    rcp32s.append(rcp32)
                for h in range(HPG):
                    b_ps = ps_s.tile([P, 512], F32, tag="s_ps", name=f"b{qc}_{h}")
                    r32 = rcp32s[h // 2]
                    off = 32 * (h % 2)
                    nc.tensor.matmul(b_ps[:], ones_r[off:off + 1, :],
                                     r32[off:off + 1, :], start=True, stop=True)
                    rcpb = asb.tile([P, 512], BF16, tag="rcpb", name=f"rb{qc}_{h}")
                    nc.vector.tensor_copy(rcpb[:], b_ps[:])
                    nc.vector.tensor_mul(attn[h][:], pvc[h][:], rcpb[:])
                def outproj(qc, attn):
                    for lsub in range(4):
                        for ec in range(4):
                            o_ps = ps_s.tile([P, 512], F32, tag="s_ps", name=f"o{qc}_{lsub}_{ec}")
                            for h in range(HPG):
                                nc.tensor.matmul(o_ps[:], attn[h][:, lsub * P:(lsub + 1) * P],
                                                 wo_sb[:, h, ec * 512:(ec + 1) * 512],
                                                 start=(h == 0), stop=(h == HPG - 1))
                            o_sb = aop.tile([P, 512], F32, tag="o_sb", name=f"ob{qc}_{lsub}_{ec}")
                            nc.vector.tensor_copy(o_sb[:], o_ps[:])
                            nc.sync.dma_start(
                                out[qc * 512 + lsub * P: qc * 512 + (lsub + 1) * P,
                                    ec * 512:(ec + 1) * 512], o_sb[:])
                pending.append((qc, attn))
                if len(pending) > 1:
                    outproj(*pending.pop(0))
            for item in pending:
                outproj(*item)
            att_ctx.close()
    nc.compile()
    return nc


# head-dim permutation: within each head, evens first then odds
_PERM = np.empty(GD, np.int64)
for _i in range(GD):
    _h, _j = divmod(_i, D)
    _par, _dd = divmod(_j, D // 2)
    _PERM[_i] = _h * D + 2 * _dd + _par


def _prepare(inputs):
    f32 = np.float32
    inputs_q = np.asarray(inputs["inputs_q"], f32)
    inputs_kv = np.asarray(inputs["inputs_kv"], f32)
    bias = np.asarray(inputs["bias"], f32).reshape(L, L)
    q_sin = np.asarray(inputs["q_sinusoids"], f32)
    k_sin = np.asarray(inputs["k_sinusoids"], f32)
    Wq = np.asarray(inputs["Wq"], f32)
    Wk = np.asarray(inputs["Wk"], f32)
    Wv = np.asarray(inputs["Wv"], f32)
    Wo = np.asarray(inputs["Wo"], f32)
    qns = np.asarray(inputs["q_norm_scale"], f32)
    kns = np.asarray(inputs["k_norm_scale"], f32)
    ls = np.asarray(inputs["logit_scale"], f32)

    apply_qs = not np.all(qns == 1.0)
    apply_ks = not np.all(kns == 1.0)

    bm = bias.max(axis=1, keepdims=True)
    expBT = np.ascontiguousarray(np.exp((bias - bm).T).astype(NP_BF16))
    ls_e = np.exp(np.minimum(ls, LOGIT_SCALE_MAX)).astype(f32)

    per_b = []
    for b in range(B):
        per_b.append(dict(
            xqT=np.ascontiguousarray(inputs_q[b].T.astype(NP_BF16)),
            xkvT=np.ascontiguousarray(inputs_kv[b].T.astype(NP_BF16)),
            c4q=np.ascontiguousarray(np.tile(q_sin[b][:, 0::2], (1, HPG))),
            s4q=np.ascontiguousarray(np.tile(q_sin[b][:, 1::2], (1, HPG))),
            c4k=np.ascontiguousarray(np.tile(k_sin[b][:, 0::2], (1, HPG))),
            s4k=np.ascontiguousarray(np.tile(k_sin[b][:, 1::2], (1, HPG))),
        ))
    per_g = []
    for g in range(G):
        rows = slice(g * GD, (g + 1) * GD)
        per_g.append(dict(
            wqT=np.ascontiguousarray(Wq[rows, :][_PERM, :].T.astype(NP_BF16)),
            wkT=np.ascontiguousarray(Wk[rows, :][_PERM, :].T.astype(NP_BF16)),
            wvT=np.ascontiguousarray(Wv[rows, :].T.astype(NP_BF16)),
            woS=np.ascontiguousarray(Wo[:, rows].T.astype(NP_BF16)),
            ls=np.broadcast_to(ls_e[g * HPG:(g + 1) * HPG][None, :], (P, HPG)).copy(),
        ))

    qs_bc = (np.broadcast_to(np.tile(qns, HPG)[_PERM][None, :], (P, GD)).copy()
             if apply_qs else None)
    ks_bc = (np.broadcast_to(np.tile(kns, HPG)[_PERM][None, :], (P, GD)).copy()
             if apply_ks else None)

    in_maps = []
    for c in range(8):
        b, g = divmod(c, G)
        m = dict(expBT=expBT)
        m.update(per_b[b])
        m.update(per_g[g])
        if apply_qs:
            m['qscale'] = qs_bc
        if apply_ks:
            m['kscale'] = ks_bc
        in_maps.append(m)
    return in_maps, apply_qs, apply_ks


_CACHE = {}


def _get_nc(apply_qs, apply_ks):
    key = (apply_qs, apply_ks)
    if key not in _CACHE:
        _CACHE[key] = _build(apply_qs, apply_ks)
    return _CACHE[key]


def kernel(**inputs) -> np.ndarray:
    in_maps, apply_qs, apply_ks = _prepare(inputs)
    nc = _get_nc(apply_qs, apply_ks)
    res = run_bass_kernel_spmd(nc, in_maps, core_ids=list(range(8)))
    out = np.zeros((B, L, E), np.float32)
    for c in range(8):
        b = c // G
        out[b] += res.results[c]["out"]
    return out
